# revision 1
# baseline (speedup 1.0000x reference)
"""Trainium2 Bass kernel for GatedSkipFusion (gate conv -> sigmoid blend ->
4-step LIF -> BatchNorm with training stats).

Self-contained: hardcodes shapes T=4, B=8, C=64, H=W=112; shards batch B
across 8 NeuronCores; BN stats via a 64-float AllReduce.

Math used:
  gate = sigmoid(pre), fused = gate*dec + (1-gate)*enc
        = enc + tanh(0.5*pre)* (0.5*(dec-enc)) ... here: enc + h*D with
          h = tanh(0.5*(pre+bg)), D = dec-enc, and the 0.5 folded into h*D
          via fused = enc + (sigma-0.5)*D + 0.5*D?  Simpler identity used:
          sigma(z) - 0.5 = 0.5*tanh(z/2)  =>  fused = 0.5*(dec+enc) + 0.5*tanh(z/2)*(dec-enc)
  We instead keep: fused = enc + sigma(z)*D. With h = tanh(z/2):
          sigma(z)*D = (0.5 + 0.5*h)*D = 0.5*D + 0.5*h*D
  To avoid extra ops we use gD = h * D2 with D2 = dec-enc and then
          fused = enc + 0.5*(D2 + gD) -- folded into the LIF update below.
  LIF (tau=2, hard reset, v_th=0.15): v_t = 0.5*v_{t-1}*m_{t-1} + fused_t,
          m = (v < th) (non-spike mask), spike s = 1-m.
  Spikes are binary => BN var = mu - mu^2; output = a*s + (beta - mu*a),
  a = gamma*rsqrt(var+eps): a per-channel affine of the spike record.
  We store sg = Sign(v - th) in {-1,0,1}; out = (a/2)*sg + (a/2 + beta - mu*a).
"""

import numpy as np

T, B, C, H, W = 4, 8, 64, 112, 112
NPIX = H * W          # 12544
BL = 448              # pixel block (free dim)
NPAIR = NPIX // (2 * BL)   # 14 pairs of blocks
NTILE = NPAIR * T     # 56 (pair,t) tiles
TH = 0.15
EPS = 1e-5
NCORES = 8
N_TOTAL = T * B * NPIX     # 401408 per-channel element count
N_CORE = T * NPIX          # 50176 per-core per-channel count

_cache = {}


def _build(reps=1, use_collective=True, num_devices=NCORES, skip=frozenset()):
    import concourse.bass as bass
    import concourse.bacc as bacc
    import concourse.mybir as mybir
    import concourse.tile as tile

    F32 = mybir.dt.float32
    BF16 = mybir.dt.bfloat16
    AF = mybir.ActivationFunctionType
    OP = mybir.AluOpType
    AX = mybir.AxisListType

    nc = bacc.Bacc("TRN2", target_bir_lowering=False, debug=False,
                   enable_asserts=False, num_devices=num_devices)

    dec_d = nc.dram_tensor("dec", [T, C, NPIX], F32, kind="ExternalInput")
    enc_d = nc.dram_tensor("enc", [T, C, NPIX], F32, kind="ExternalInput")
    wd_d = nc.dram_tensor("wd", [128, 128], F32, kind="ExternalInput")
    we_d = nc.dram_tensor("we", [128, 128], F32, kind="ExternalInput")
    bgh_d = nc.dram_tensor("bgh", [128, 1], F32, kind="ExternalInput")
    nth_d = nc.dram_tensor("nth", [128, 1], F32, kind="ExternalInput")
    gam_d = nc.dram_tensor("gam", [64, 1], F32, kind="ExternalInput")
    bet_d = nc.dram_tensor("bet", [64, 1], F32, kind="ExternalInput")
    out_d = nc.dram_tensor("out", [T, C, NPIX], F32, kind="ExternalOutput")

    with tile.TileContext(nc) as tc:
        with tc.tile_pool(name="const", bufs=1) as cp, \
             tc.tile_pool(name="io", bufs=2) as io, \
             tc.tile_pool(name="wk", bufs=2) as wk, \
             tc.tile_pool(name="vv", bufs=4) as vv, \
             tc.tile_pool(name="sm", bufs=6) as sm, \
             tc.tile_pool(name="ps", bufs=2, space="PSUM") as ps, \
             tc.tile_pool(name="dram", bufs=2, space="DRAM") as dp:

            wd_t = cp.tile([128, 128], F32)
            we_t = cp.tile([128, 128], F32)
            bgh_t = cp.tile([128, 1], F32)
            nth_t = cp.tile([128, 1], F32)
            gam_t = cp.tile([64, 1], F32)
            bet_t = cp.tile([64, 1], F32)
            nc.sync.dma_start(wd_t[:], wd_d[:, :])
            nc.sync.dma_start(we_t[:], we_d[:, :])
            nc.sync.dma_start(bgh_t[:], bgh_d[:, :])
            nc.sync.dma_start(nth_t[:], nth_d[:, :])
            nc.sync.dma_start(gam_t[:], gam_d[:, :])
            nc.sync.dma_start(bet_t[:], bet_d[:, :])

            store = cp.tile([128, BL * NTILE], BF16)  # sign record
            scol = cp.tile([128, NTILE], F32)         # per-tile sign sums

            for _rep in range(reps):
                # ---------------- pass 1 ----------------
                # dec4/enc4: [128, 4*448]: partitions = (block, channel),
                # free = (t, pixel-in-block); one DMA each per pair.
                dec_r = dec_d.rearrange("t c (blk x) -> blk c t x", blk=28)
                enc_r = enc_d.rearrange("t c (blk x) -> blk c t x", blk=28)
                for pair in range(NPAIR):
                    vr = None
                    dec4 = io.tile([128, T * BL], F32)
                    enc4 = io.tile([128, T * BL], F32)
                    # partitions 0:64 = block 2*pair (channels), 64:128 = block 2*pair+1
                    nc.sync.dma_start(dec4[0:64, :], dec_r[2 * pair])
                    nc.sync.dma_start(dec4[64:128, :], dec_r[2 * pair + 1])
                    nc.sync.dma_start(enc4[0:64, :], enc_r[2 * pair])
                    nc.sync.dma_start(enc4[64:128, :], enc_r[2 * pair + 1])

                    # t-independent wide ops
                    D4 = wk.tile([128, T * BL], F32)
                    nc.vector.tensor_tensor(D4[:], dec4[:], enc4[:], OP.subtract)
                    h4 = wk.tile([128, T * BL], F32)
                    for t in range(T):
                        sl4 = slice(t * BL, (t + 1) * BL)
                        pp = ps.tile([128, BL], F32)
                        nc.tensor.matmul(out=pp[:], lhsT=wd_t[:],
                                         rhs=dec4[:, sl4],
                                         start=True, stop=False)
                        nc.tensor.matmul(out=pp[:], lhsT=we_t[:],
                                         rhs=enc4[:, sl4],
                                         start=False, stop=True)
                        nc.scalar.activation(h4[:, sl4], pp[:], AF.Tanh,
                                             bias=bgh_t[:], scale=0.5)
                    gD4 = wk.tile([128, T * BL], F32)
                    nc.vector.scalar_tensor_tensor(out=gD4[:], in0=h4[:],
                                                   scalar=1.0, in1=D4[:],
                                                   op0=OP.add, op1=OP.mult)
                    # F4 = fused for all t: 0.5*gD4 + enc  (wide, t-independent)
                    F4 = wk.tile([128, T * BL], F32)
                    nc.vector.scalar_tensor_tensor(out=F4[:], in0=gD4[:],
                                                   scalar=0.5, in1=enc4[:],
                                                   op0=OP.mult, op1=OP.add)
                    # per-t LIF recurrence: v' = 0.5*vr + F
                    for t in range(T):
                        idx = pair * T + t
                        sl4 = slice(t * BL, (t + 1) * BL)
                        if t == 0:
                            vp = F4[:, sl4]
                        else:
                            vpt = sm.tile([128, BL], F32)
                            nc.vector.scalar_tensor_tensor(
                                out=vpt[:], in0=vr[:], scalar=0.5,
                                in1=F4[:, sl4], op0=OP.mult, op1=OP.add)
                            vp = vpt[:]
                        sl = store[:, idx * BL:(idx + 1) * BL]
                        nc.scalar.activation(sl, vp, AF.Sign, bias=nth_t[:],
                                             scale=1.0,
                                             accum_out=scol[:, idx:idx + 1])
                        if t < T - 1:
                            vrn = vv.tile([128, BL], F32)
                            nc.vector.scalar_tensor_tensor(
                                out=vrn[:], in0=vp, scalar=TH, in1=vp,
                                op0=OP.is_lt, op1=OP.mult)
                            vr = vrn

            # ---------------- stats ----------------
                red = cp.tile([128, 1], F32)
                nc.vector.tensor_reduce(out=red[:], in_=scol[:, 0:NTILE],
                                        axis=AX.X, op=OP.add)
                redB = cp.tile([64, 1], F32)
                nc.sync.dma_start(redB[:], red[64:128, :])
                s64 = cp.tile([64, 1], F32)
                nc.vector.tensor_tensor(s64[:], red[0:64, :], redB[:], OP.add)
                # local spike count = 0.5*sum_sign + N_CORE/2
                loc = cp.tile([64, 1], F32)
                nc.vector.tensor_scalar(out=loc[:], in0=s64[:], scalar1=0.5,
                                        scalar2=float(N_CORE) / 2.0,
                                        op0=OP.mult, op1=OP.add)
                cin = dp.tile([64, 1], F32)
                cout = dp.tile([64, 1], F32)
                S = cp.tile([64, 1], F32)
                if use_collective:
                    nc.sync.dma_start(cin[:], loc[:])
                    nc.gpsimd.collective_compute(
                        "AllReduce", OP.add,
                        replica_groups=[list(range(num_devices))],
                        ins=[cin.opt()], outs=[cout.opt()])
                    nc.sync.dma_start(S[:], cout[:])
                else:
                    nc.vector.tensor_scalar(out=S[:], in0=loc[:],
                                            scalar1=float(NCORES),
                                            scalar2=None, op0=OP.mult)

                mu = cp.tile([64, 1], F32)
                nc.vector.tensor_scalar(out=mu[:], in0=S[:],
                                        scalar1=1.0 / float(N_TOTAL), scalar2=None,
                                        op0=OP.mult)
                # x = mu - mu^2 + eps
                m2 = cp.tile([64, 1], F32)
                nc.vector.tensor_tensor(m2[:], mu[:], mu[:], OP.mult)
                x = cp.tile([64, 1], F32)
                nc.vector.tensor_tensor(x[:], mu[:], m2[:], OP.subtract)
                nc.vector.tensor_scalar(out=x[:], in0=x[:], scalar1=EPS,
                                        scalar2=None, op0=OP.add)
                # r ~= rsqrt(x) then 2 Newton iterations r *= 1.5 - 0.5*x*r^2
                sq = cp.tile([64, 1], F32)
                nc.scalar.activation(sq[:], x[:], AF.Sqrt)
                r = cp.tile([64, 1], F32)
                nc.vector.reciprocal(r[:], sq[:])
                for _ in range(2):
                    e = cp.tile([64, 1], F32)
                    nc.vector.tensor_tensor(e[:], r[:], r[:], OP.mult)
                    nc.vector.tensor_tensor(e[:], e[:], x[:], OP.mult)
                    nc.vector.tensor_scalar(out=e[:], in0=e[:], scalar1=-0.5,
                                            scalar2=1.5, op0=OP.mult, op1=OP.add)
                    nc.vector.tensor_tensor(r[:], r[:], e[:], OP.mult)
                # a = gamma*r ; scale = a/2 ; bias = a/2 + beta - mu*a
                a = cp.tile([64, 1], F32)
                nc.vector.tensor_tensor(a[:], gam_t[:], r[:], OP.mult)
                sc64 = cp.tile([64, 1], F32)
                nc.vector.tensor_scalar(out=sc64[:], in0=a[:], scalar1=0.5,
                                        scalar2=None, op0=OP.mult)
                tmp = cp.tile([64, 1], F32)
                nc.vector.tensor_tensor(tmp[:], mu[:], a[:], OP.mult)
                b0 = cp.tile([64, 1], F32)
                nc.vector.tensor_tensor(b0[:], bet_t[:], tmp[:], OP.subtract)
                bi64 = cp.tile([64, 1], F32)
                nc.vector.tensor_tensor(bi64[:], sc64[:], b0[:], OP.add)
                sc128 = cp.tile([128, 1], F32)
                bi128 = cp.tile([128, 1], F32)
                nc.sync.dma_start(sc128[0:64, :], sc64[:])
                nc.sync.dma_start(sc128[64:128, :], sc64[:])
                nc.sync.dma_start(bi128[0:64, :], bi64[:])
                nc.sync.dma_start(bi128[64:128, :], bi64[:])

                # ---------------- pass 2 ----------------
                out_r = out_d.rearrange("t c (blk x) -> blk c t x", blk=28)
                for pair in range(NPAIR):
                    ot = io.tile([128, T * BL], F32)
                    nc.scalar.activation(
                        ot[:], store[:, pair * T * BL:(pair + 1) * T * BL],
                        AF.Identity, bias=bi128[:], scale=sc128[:])
                    nc.sync.dma_start(out_r[2 * pair], ot[0:64, :])
                    nc.sync.dma_start(out_r[2 * pair + 1], ot[64:128, :])

    nc.compile()
    return nc


def _prep_host(dec, enc, Wg, bg, gamma, beta):
    Wg = np.asarray(Wg, dtype=np.float32)
    wdT = np.ascontiguousarray(Wg[:, :64].T)   # [k, m] dec-part
    weT = np.ascontiguousarray(Wg[:, 64:].T)   # enc-part
    wd = np.zeros((128, 128), dtype=np.float32)
    we = np.zeros((128, 128), dtype=np.float32)
    wd[:64, :64] = wdT
    wd[64:, 64:] = wdT
    we[:64, :64] = weT
    we[64:, 64:] = weT
    bgh = np.tile(0.5 * np.asarray(bg, np.float32), 2).reshape(128, 1)
    nth = np.full((128, 1), -TH, dtype=np.float32)
    gam = np.asarray(gamma, np.float32).reshape(64, 1)
    bet = np.asarray(beta, np.float32).reshape(64, 1)
    in_maps = []
    for b in range(NCORES):
        in_maps.append({
            "dec": np.ascontiguousarray(dec[:, b]).reshape(T, C, NPIX),
            "enc": np.ascontiguousarray(enc[:, b]).reshape(T, C, NPIX),
            "wd": wd, "we": we, "bgh": bgh, "nth": nth,
            "gam": gam, "bet": bet,
        })
    return in_maps


def kernel(dec, enc, Wg, bg, gamma, beta, _trace=False, _trace_kwargs=None):
    from concourse.bass_utils import run_bass_kernel_spmd

    if "nc" not in _cache:
        _cache["nc"] = _build()
    nc = _cache["nc"]

    in_maps = _prep_host(dec, enc, Wg, bg, gamma, beta)
    kw = {}
    if _trace:
        kw["trace"] = True
        if _trace_kwargs:
            kw.update(_trace_kwargs)
    res = run_bass_kernel_spmd(nc, in_maps, core_ids=list(range(NCORES)), **kw)
    out = np.stack([res.results[b]["out"] for b in range(NCORES)], axis=1)
    out = out.reshape(T, B, C, H, W)
    if _trace:
        _cache["last_res"] = res
    return out



# revision 3
# speedup vs baseline: 1.0367x; 1.0367x over previous
"""Trainium2 Bass kernel for GatedSkipFusion (gate conv -> sigmoid blend ->
4-step LIF -> BatchNorm with training stats).

Self-contained: hardcodes shapes T=4, B=8, C=64, H=W=112; shards batch B
across 8 NeuronCores; BN stats via a 64-float AllReduce.

Math used:
  gate = sigmoid(pre), fused = gate*dec + (1-gate)*enc
        = enc + tanh(0.5*pre)* (0.5*(dec-enc)) ... here: enc + h*D with
          h = tanh(0.5*(pre+bg)), D = dec-enc, and the 0.5 folded into h*D
          via fused = enc + (sigma-0.5)*D + 0.5*D?  Simpler identity used:
          sigma(z) - 0.5 = 0.5*tanh(z/2)  =>  fused = 0.5*(dec+enc) + 0.5*tanh(z/2)*(dec-enc)
  We instead keep: fused = enc + sigma(z)*D. With h = tanh(z/2):
          sigma(z)*D = (0.5 + 0.5*h)*D = 0.5*D + 0.5*h*D
  To avoid extra ops we use gD = h * D2 with D2 = dec-enc and then
          fused = enc + 0.5*(D2 + gD) -- folded into the LIF update below.
  LIF (tau=2, hard reset, v_th=0.15): v_t = 0.5*v_{t-1}*m_{t-1} + fused_t,
          m = (v < th) (non-spike mask), spike s = 1-m.
  Spikes are binary => BN var = mu - mu^2; output = a*s + (beta - mu*a),
  a = gamma*rsqrt(var+eps): a per-channel affine of the spike record.
  We store sg = Sign(v - th) in {-1,0,1}; out = (a/2)*sg + (a/2 + beta - mu*a).
"""

import numpy as np

T, B, C, H, W = 4, 8, 64, 112, 112
NPIX = H * W          # 12544
BL = 448              # pixel block (free dim)
NPAIR = NPIX // (2 * BL)   # 14 pairs of blocks
NTILE = NPAIR * T     # 56 (pair,t) tiles
TH = 0.15
EPS = 1e-5
NCORES = 8
N_TOTAL = T * B * NPIX     # 401408 per-channel element count
N_CORE = T * NPIX          # 50176 per-core per-channel count

_cache = {}


def _build(reps=1, use_collective=True, num_devices=NCORES, skip=frozenset()):
    import concourse.bass as bass
    import concourse.bacc as bacc
    import concourse.mybir as mybir
    import concourse.tile as tile

    F32 = mybir.dt.float32
    F32R = mybir.dt.float32r
    BF16 = mybir.dt.bfloat16
    AF = mybir.ActivationFunctionType
    OP = mybir.AluOpType
    AX = mybir.AxisListType

    nc = bacc.Bacc("TRN2", target_bir_lowering=False, debug=False,
                   enable_asserts=False, num_devices=num_devices)

    dec_d = nc.dram_tensor("dec", [T, C, NPIX], F32R, kind="ExternalInput")
    enc_d = nc.dram_tensor("enc", [T, C, NPIX], F32R, kind="ExternalInput")
    wd_d = nc.dram_tensor("wd", [128, 128], F32R, kind="ExternalInput")
    we_d = nc.dram_tensor("we", [128, 128], F32R, kind="ExternalInput")
    bgh_d = nc.dram_tensor("bgh", [128, 1], F32, kind="ExternalInput")
    nth_d = nc.dram_tensor("nth", [128, 1], F32, kind="ExternalInput")
    gam_d = nc.dram_tensor("gam", [64, 1], F32, kind="ExternalInput")
    bet_d = nc.dram_tensor("bet", [64, 1], F32, kind="ExternalInput")
    out_d = nc.dram_tensor("out", [T, C, NPIX], F32, kind="ExternalOutput")

    with tile.TileContext(nc) as tc:
        with tc.tile_pool(name="const", bufs=1) as cp, \
             tc.tile_pool(name="io", bufs=2) as io, \
             tc.tile_pool(name="wk", bufs=2) as wk, \
             tc.tile_pool(name="vv", bufs=4) as vv, \
             tc.tile_pool(name="sm", bufs=6) as sm, \
             tc.tile_pool(name="ps", bufs=2, space="PSUM") as ps, \
             tc.tile_pool(name="dram", bufs=2, space="DRAM") as dp:

            wd_t = cp.tile([128, 128], F32R)
            we_t = cp.tile([128, 128], F32R)
            bgh_t = cp.tile([128, 1], F32)
            nth_t = cp.tile([128, 1], F32)
            gam_t = cp.tile([64, 1], F32)
            bet_t = cp.tile([64, 1], F32)
            nc.sync.dma_start(wd_t[:], wd_d[:, :])
            nc.sync.dma_start(we_t[:], we_d[:, :])
            nc.sync.dma_start(bgh_t[:], bgh_d[:, :])
            nc.sync.dma_start(nth_t[:], nth_d[:, :])
            nc.sync.dma_start(gam_t[:], gam_d[:, :])
            nc.sync.dma_start(bet_t[:], bet_d[:, :])

            store = cp.tile([128, BL * NTILE], BF16)  # sign record
            scol = cp.tile([128, NTILE], F32)         # per-tile sign sums

            for _rep in range(reps):
                # ---------------- pass 1 ----------------
                # dec4/enc4: [128, 4*448]: partitions = (block, channel),
                # free = (t, pixel-in-block); one DMA each per pair.
                dec_r = dec_d.rearrange("t c (blk x) -> blk c t x", blk=28)
                enc_r = enc_d.rearrange("t c (blk x) -> blk c t x", blk=28)
                for pair in range(NPAIR):
                    vr = None
                    dec4 = io.tile([128, T * BL], F32R)
                    enc4 = io.tile([128, T * BL], F32R)
                    # partitions 0:64 = block 2*pair (channels), 64:128 = block 2*pair+1
                    nc.sync.dma_start(dec4[0:64, :], dec_r[2 * pair])
                    nc.sync.dma_start(dec4[64:128, :], dec_r[2 * pair + 1])
                    nc.sync.dma_start(enc4[0:64, :], enc_r[2 * pair])
                    nc.sync.dma_start(enc4[64:128, :], enc_r[2 * pair + 1])

                    # t-independent wide ops
                    D4 = wk.tile([128, T * BL], F32)
                    nc.vector.tensor_tensor(D4[:], dec4[:].bitcast(F32), enc4[:].bitcast(F32), OP.subtract)
                    h4 = wk.tile([128, T * BL], F32)
                    for t in range(T):
                        sl4 = slice(t * BL, (t + 1) * BL)
                        pp = ps.tile([128, BL], F32)
                        nc.tensor.matmul(out=pp[:], lhsT=wd_t[:],
                                         rhs=dec4[:, sl4],
                                         start=True, stop=False)
                        nc.tensor.matmul(out=pp[:], lhsT=we_t[:],
                                         rhs=enc4[:, sl4],
                                         start=False, stop=True)
                        nc.scalar.activation(h4[:, sl4], pp[:], AF.Tanh,
                                             bias=bgh_t[:], scale=0.5)
                    gD4 = wk.tile([128, T * BL], F32)
                    nc.vector.scalar_tensor_tensor(out=gD4[:], in0=h4[:],
                                                   scalar=1.0, in1=D4[:],
                                                   op0=OP.add, op1=OP.mult)
                    # F4 = fused for all t: 0.5*gD4 + enc  (wide, t-independent)
                    F4 = wk.tile([128, T * BL], F32)
                    nc.vector.scalar_tensor_tensor(out=F4[:], in0=gD4[:],
                                                   scalar=0.5, in1=enc4[:].bitcast(F32),
                                                   op0=OP.mult, op1=OP.add)
                    # per-t LIF recurrence: v' = 0.5*vr + F
                    for t in range(T):
                        idx = pair * T + t
                        sl4 = slice(t * BL, (t + 1) * BL)
                        if t == 0:
                            vp = F4[:, sl4]
                        else:
                            vpt = sm.tile([128, BL], F32)
                            nc.vector.scalar_tensor_tensor(
                                out=vpt[:], in0=vr[:], scalar=0.5,
                                in1=F4[:, sl4], op0=OP.mult, op1=OP.add)
                            vp = vpt[:]
                        sl = store[:, idx * BL:(idx + 1) * BL]
                        nc.scalar.activation(sl, vp, AF.Sign, bias=nth_t[:],
                                             scale=1.0,
                                             accum_out=scol[:, idx:idx + 1])
                        if t < T - 1:
                            vrn = vv.tile([128, BL], F32)
                            nc.vector.scalar_tensor_tensor(
                                out=vrn[:], in0=vp, scalar=TH, in1=vp,
                                op0=OP.is_lt, op1=OP.mult)
                            vr = vrn

            # ---------------- stats ----------------
                red = cp.tile([128, 1], F32)
                nc.vector.tensor_reduce(out=red[:], in_=scol[:, 0:NTILE],
                                        axis=AX.X, op=OP.add)
                redB = cp.tile([64, 1], F32)
                nc.sync.dma_start(redB[:], red[64:128, :])
                s64 = cp.tile([64, 1], F32)
                nc.vector.tensor_tensor(s64[:], red[0:64, :], redB[:], OP.add)
                # local spike count = 0.5*sum_sign + N_CORE/2
                loc = cp.tile([64, 1], F32)
                nc.vector.tensor_scalar(out=loc[:], in0=s64[:], scalar1=0.5,
                                        scalar2=float(N_CORE) / 2.0,
                                        op0=OP.mult, op1=OP.add)
                cin = dp.tile([64, 1], F32)
                cout = dp.tile([64, 1], F32)
                S = cp.tile([64, 1], F32)
                if use_collective:
                    nc.sync.dma_start(cin[:], loc[:])
                    nc.gpsimd.collective_compute(
                        "AllReduce", OP.add,
                        replica_groups=[list(range(num_devices))],
                        ins=[cin.opt()], outs=[cout.opt()])
                    nc.sync.dma_start(S[:], cout[:])
                else:
                    nc.vector.tensor_scalar(out=S[:], in0=loc[:],
                                            scalar1=float(NCORES),
                                            scalar2=None, op0=OP.mult)

                mu = cp.tile([64, 1], F32)
                nc.vector.tensor_scalar(out=mu[:], in0=S[:],
                                        scalar1=1.0 / float(N_TOTAL), scalar2=None,
                                        op0=OP.mult)
                # x = mu - mu^2 + eps
                m2 = cp.tile([64, 1], F32)
                nc.vector.tensor_tensor(m2[:], mu[:], mu[:], OP.mult)
                x = cp.tile([64, 1], F32)
                nc.vector.tensor_tensor(x[:], mu[:], m2[:], OP.subtract)
                nc.vector.tensor_scalar(out=x[:], in0=x[:], scalar1=EPS,
                                        scalar2=None, op0=OP.add)
                # r ~= rsqrt(x) then 2 Newton iterations r *= 1.5 - 0.5*x*r^2
                sq = cp.tile([64, 1], F32)
                nc.scalar.activation(sq[:], x[:], AF.Sqrt)
                r = cp.tile([64, 1], F32)
                nc.vector.reciprocal(r[:], sq[:])
                for _ in range(2):
                    e = cp.tile([64, 1], F32)
                    nc.vector.tensor_tensor(e[:], r[:], r[:], OP.mult)
                    nc.vector.tensor_tensor(e[:], e[:], x[:], OP.mult)
                    nc.vector.tensor_scalar(out=e[:], in0=e[:], scalar1=-0.5,
                                            scalar2=1.5, op0=OP.mult, op1=OP.add)
                    nc.vector.tensor_tensor(r[:], r[:], e[:], OP.mult)
                # a = gamma*r ; scale = a/2 ; bias = a/2 + beta - mu*a
                a = cp.tile([64, 1], F32)
                nc.vector.tensor_tensor(a[:], gam_t[:], r[:], OP.mult)
                sc64 = cp.tile([64, 1], F32)
                nc.vector.tensor_scalar(out=sc64[:], in0=a[:], scalar1=0.5,
                                        scalar2=None, op0=OP.mult)
                tmp = cp.tile([64, 1], F32)
                nc.vector.tensor_tensor(tmp[:], mu[:], a[:], OP.mult)
                b0 = cp.tile([64, 1], F32)
                nc.vector.tensor_tensor(b0[:], bet_t[:], tmp[:], OP.subtract)
                bi64 = cp.tile([64, 1], F32)
                nc.vector.tensor_tensor(bi64[:], sc64[:], b0[:], OP.add)
                sc128 = cp.tile([128, 1], F32)
                bi128 = cp.tile([128, 1], F32)
                nc.sync.dma_start(sc128[0:64, :], sc64[:])
                nc.sync.dma_start(sc128[64:128, :], sc64[:])
                nc.sync.dma_start(bi128[0:64, :], bi64[:])
                nc.sync.dma_start(bi128[64:128, :], bi64[:])

                # ---------------- pass 2 ----------------
                out_r = out_d.rearrange("t c (blk x) -> blk c t x", blk=28)
                for pair in range(NPAIR):
                    ot = io.tile([128, T * BL], F32)
                    nc.scalar.activation(
                        ot[:], store[:, pair * T * BL:(pair + 1) * T * BL],
                        AF.Identity, bias=bi128[:], scale=sc128[:])
                    nc.sync.dma_start(out_r[2 * pair], ot[0:64, :])
                    nc.sync.dma_start(out_r[2 * pair + 1], ot[64:128, :])

    nc.compile()
    return nc


def _prep_host(dec, enc, Wg, bg, gamma, beta):
    Wg = np.asarray(Wg, dtype=np.float32)
    wdT = np.ascontiguousarray(Wg[:, :64].T)   # [k, m] dec-part
    weT = np.ascontiguousarray(Wg[:, 64:].T)   # enc-part
    wd = np.zeros((128, 128), dtype=np.float32)
    we = np.zeros((128, 128), dtype=np.float32)
    wd[:64, :64] = wdT
    wd[64:, 64:] = wdT
    we[:64, :64] = weT
    we[64:, 64:] = weT
    bgh = np.tile(0.5 * np.asarray(bg, np.float32), 2).reshape(128, 1)
    nth = np.full((128, 1), -TH, dtype=np.float32)
    gam = np.asarray(gamma, np.float32).reshape(64, 1)
    bet = np.asarray(beta, np.float32).reshape(64, 1)
    in_maps = []
    for b in range(NCORES):
        in_maps.append({
            "dec": np.ascontiguousarray(dec[:, b]).reshape(T, C, NPIX),
            "enc": np.ascontiguousarray(enc[:, b]).reshape(T, C, NPIX),
            "wd": wd, "we": we, "bgh": bgh, "nth": nth,
            "gam": gam, "bet": bet,
        })
    return in_maps


def kernel(dec, enc, Wg, bg, gamma, beta, _trace=False, _trace_kwargs=None):
    from concourse.bass_utils import run_bass_kernel_spmd

    if "nc" not in _cache:
        _cache["nc"] = _build()
    nc = _cache["nc"]

    in_maps = _prep_host(dec, enc, Wg, bg, gamma, beta)
    kw = {}
    if _trace:
        kw["trace"] = True
        if _trace_kwargs:
            kw.update(_trace_kwargs)
    res = run_bass_kernel_spmd(nc, in_maps, core_ids=list(range(NCORES)), **kw)
    out = np.stack([res.results[b]["out"] for b in range(NCORES)], axis=1)
    out = out.reshape(T, B, C, H, W)
    if _trace:
        _cache["last_res"] = res
    return out



# revision 14
# speedup vs baseline: 1.1141x; 1.0746x over previous
"""Trainium2 Bass kernel for GatedSkipFusion (gate conv -> sigmoid blend ->
4-step LIF -> BatchNorm with training stats).

Self-contained: hardcodes shapes T=4, B=8, C=64, H=W=112; shards batch B
across 8 NeuronCores; BN stats via a 64-float AllReduce.

Math:
  gate = sigmoid(pre); fused = enc + gate*(dec-enc). With h = tanh(pre/2):
  gate = 0.5 + 0.5*h, so fused = enc + 0.5*(1+h)*D, D = dec-enc.
  LIF (tau=2, hard reset, v_th=0.15): v_t = 0.5*v_{t-1}*m_{t-1} + fused_t,
  m = (v < th). Spikes are binary so BN var = mu - mu^2; the BN output is a
  per-channel affine of the sign record sg = Sign(v - th) in {-1,0,1}:
  out = (a/2)*sg + (a/2 + beta - mu*a), a = gamma*rsqrt(var+eps).

Engine split (all four compute engines + DMA overlap; the per-core program
is memory-bound at ~360 GB/s):
  PE    : gate matmuls in fp32r (1 cyc/row)
  Act   : batched tanh from a 4-bank PSUM tile; per-step Sign with
          accumulation for the BN statistics
  DVE   : gD=(1+h)*D, F=0.5*gD+enc, reset-mask mult, and the final
          affine as a 4x-mode fp16 tensor_scalar
  Pool  : D=dec-enc and the v-update scalar_tensor_tensor
  fp16 output (halves the output DMA; ~6e-4 systematic error).
"""

import numpy as np

T, B, C, H, W = 4, 8, 64, 112, 112
NPIX = H * W          # 12544
BL = 448              # pixel block (free dim)
NPAIR = NPIX // (2 * BL)   # 14 pairs of blocks
NTILE = NPAIR * T     # 56 (pair,t) tiles
TH = 0.15
EPS = 1e-5
NCORES = 8
N_TOTAL = T * B * NPIX     # 401408 per-channel element count
N_CORE = T * NPIX          # 50176 per-core per-channel count

_cache = {}


def _build(reps=1, use_collective=True, num_devices=NCORES, d_on_pe=True,
           skip=frozenset()):
    import concourse.bass as bass
    import concourse.bacc as bacc
    import concourse.mybir as mybir
    import concourse.tile as tile

    F32 = mybir.dt.float32
    F32R = mybir.dt.float32r
    F16 = mybir.dt.float16
    AF = mybir.ActivationFunctionType
    OP = mybir.AluOpType
    AX = mybir.AxisListType

    nc = bacc.Bacc("TRN2", target_bir_lowering=False, debug=False,
                   enable_asserts=False, num_devices=num_devices)

    dec_d = nc.dram_tensor("dec", [T, C, NPIX], F32R, kind="ExternalInput")
    enc_d = nc.dram_tensor("enc", [T, C, NPIX], F32R, kind="ExternalInput")
    wd_d = nc.dram_tensor("wd", [128, 128], F32R, kind="ExternalInput")
    we_d = nc.dram_tensor("we", [128, 128], F32R, kind="ExternalInput")
    idp_d = nc.dram_tensor("idp", [128, 128], F32R, kind="ExternalInput")
    idm_d = nc.dram_tensor("idm", [128, 128], F32R, kind="ExternalInput")
    bgh_d = nc.dram_tensor("bgh", [128, 1], F32, kind="ExternalInput")
    nth_d = nc.dram_tensor("nth", [128, 1], F32, kind="ExternalInput")
    gam_d = nc.dram_tensor("gam", [64, 1], F32, kind="ExternalInput")
    bet_d = nc.dram_tensor("bet", [64, 1], F32, kind="ExternalInput")
    out_d = nc.dram_tensor("out", [T, C, NPIX], F16, kind="ExternalOutput")

    with tile.TileContext(nc) as tc:
        with tc.tile_pool(name="const", bufs=1) as cp, \
             tc.tile_pool(name="io", bufs=3) as io, \
             tc.tile_pool(name="wk", bufs=2) as wk, \
             tc.tile_pool(name="vv", bufs=4) as vv, \
             tc.tile_pool(name="sm", bufs=6) as sm, \
             tc.tile_pool(name="ot", bufs=2) as op_, \
             tc.tile_pool(name="ps", bufs=1, space="PSUM") as ps, \
             tc.tile_pool(name="psd", bufs=1, space="PSUM") as psd, \
             tc.tile_pool(name="dram", bufs=2, space="DRAM") as dp:

            wd_t = cp.tile([128, 128], F32R)
            we_t = cp.tile([128, 128], F32R)
            idp_t = cp.tile([128, 128], F32R)
            idm_t = cp.tile([128, 128], F32R)
            nc.sync.dma_start(idp_t[:], idp_d[:, :])
            nc.sync.dma_start(idm_t[:], idm_d[:, :])
            bgh_t = cp.tile([128, 1], F32)
            nth_t = cp.tile([128, 1], F32)
            gam_t = cp.tile([64, 1], F32)
            bet_t = cp.tile([64, 1], F32)
            nc.sync.dma_start(wd_t[:], wd_d[:, :])
            nc.sync.dma_start(we_t[:], we_d[:, :])
            nc.sync.dma_start(bgh_t[:], bgh_d[:, :])
            nc.sync.dma_start(nth_t[:], nth_d[:, :])
            nc.sync.dma_start(gam_t[:], gam_d[:, :])
            nc.sync.dma_start(bet_t[:], bet_d[:, :])

            store = cp.tile([128, NTILE, BL], F16)    # sign record
            scol = cp.tile([128, NTILE], F32)         # per-tile sign sums

            dec_r = dec_d.rearrange("t c (blk x) -> blk c t x", blk=28)
            enc_r = enc_d.rearrange("t c (blk x) -> blk c t x", blk=28)
            out_r = out_d.rearrange("t c (blk x) -> blk c t x", blk=28)

            for _rep in range(reps):
                # ---------------- pass 1 ----------------
                for pair in range(NPAIR):
                    dec4 = io.tile([128, T, BL], F32R)
                    enc4 = io.tile([128, T, BL], F32R)
                    nc.sync.dma_start(dec4[0:64], dec_r[2 * pair])
                    nc.sync.dma_start(dec4[64:128], dec_r[2 * pair + 1])
                    nc.sync.dma_start(enc4[0:64], enc_r[2 * pair])
                    nc.sync.dma_start(enc4[64:128], enc_r[2 * pair + 1])

                    # gate logits for all t into one 4-bank PSUM tile
                    P4 = ps.tile([128, T, 512], F32)
                    for t in range(T):
                        nc.tensor.matmul(out=P4[:, t, 0:BL], lhsT=wd_t[:],
                                         rhs=dec4[:, t], start=True,
                                         stop=False)
                        nc.tensor.matmul(out=P4[:, t, 0:BL], lhsT=we_t[:],
                                         rhs=enc4[:, t], start=False,
                                         stop=True)
                    h4 = wk.tile([128, T, BL], F32)
                    nc.scalar.activation(h4[:], P4[:, :, 0:BL], AF.Tanh,
                                         bias=bgh_t[:], scale=0.5)

                    if d_on_pe:
                        # Dh = 0.5*(dec - enc) via +-0.5*I fp32r matmuls
                        D4ap = psd.tile([128, T, 512], F32)
                        for t in range(T):
                            nc.tensor.matmul(out=D4ap[:, t, 0:BL],
                                             lhsT=idp_t[:], rhs=dec4[:, t],
                                             start=True, stop=False)
                            nc.tensor.matmul(out=D4ap[:, t, 0:BL],
                                             lhsT=idm_t[:], rhs=enc4[:, t],
                                             start=False, stop=True)
                        D4 = D4ap[:, :, 0:BL]
                    else:
                        D4t = wk.tile([128, T, BL], F32)
                        nc.vector.tensor_tensor(D4t[:], dec4[:].bitcast(F32),
                                                enc4[:].bitcast(F32),
                                                OP.subtract)
                        D4 = D4t[:]
                    dscale = 1.0 if d_on_pe else 0.5
                    # gD = (1+h)*Dh = sigma(pre)*(dec-enc) ; F = gD + enc
                    gD4 = wk.tile([128, T, BL], F32)
                    nc.vector.scalar_tensor_tensor(out=gD4[:], in0=h4[:],
                                                   scalar=1.0, in1=D4,
                                                   op0=OP.add, op1=OP.mult)
                    F4 = wk.tile([128, T, BL], F32)
                    nc.vector.scalar_tensor_tensor(out=F4[:], in0=gD4[:],
                                                   scalar=dscale,
                                                   in1=enc4[:].bitcast(F32),
                                                   op0=OP.mult, op1=OP.add)
                    # per-t LIF recurrence
                    vp = F4[:, 0]
                    for t in range(T):
                        idx = pair * T + t
                        nc.scalar.activation(store[:, idx], vp, AF.Sign,
                                             bias=nth_t[:], scale=1.0,
                                             accum_out=scol[:, idx:idx + 1])
                        if t < T - 1:
                            vrn = vv.tile([128, BL], F32)
                            nc.vector.scalar_tensor_tensor(
                                out=vrn[:], in0=vp, scalar=TH, in1=vp,
                                op0=OP.is_lt, op1=OP.mult)
                            vpt = sm.tile([128, BL], F32)
                            nc.vector.scalar_tensor_tensor(
                                out=vpt[:], in0=vrn[:], scalar=0.5,
                                in1=F4[:, t + 1], op0=OP.mult, op1=OP.add)
                            vp = vpt[:]

                # ---------------- stats ----------------
                red = cp.tile([128, 1], F32)
                nc.vector.tensor_reduce(out=red[:], in_=scol[:, 0:NTILE],
                                        axis=AX.X, op=OP.add)
                redB = cp.tile([64, 1], F32)
                nc.sync.dma_start(redB[:], red[64:128, :])
                s64 = cp.tile([64, 1], F32)
                nc.vector.tensor_tensor(s64[:], red[0:64, :], redB[:], OP.add)
                # local spike count = 0.5*sum_sign + N_CORE/2
                loc = cp.tile([64, 1], F32)
                nc.vector.tensor_scalar(out=loc[:], in0=s64[:], scalar1=0.5,
                                        scalar2=float(N_CORE) / 2.0,
                                        op0=OP.mult, op1=OP.add)
                cin = dp.tile([64, 1], F32)
                cout = dp.tile([64, 1], F32)
                S = cp.tile([64, 1], F32)
                if use_collective:
                    nc.sync.dma_start(cin[:], loc[:])
                    nc.gpsimd.collective_compute(
                        "AllReduce", OP.add,
                        replica_groups=[list(range(num_devices))],
                        ins=[cin.opt()], outs=[cout.opt()])
                    nc.sync.dma_start(S[:], cout[:])
                else:
                    nc.vector.tensor_scalar(out=S[:], in0=loc[:],
                                            scalar1=float(NCORES),
                                            scalar2=None, op0=OP.mult)

                mu = cp.tile([64, 1], F32)
                nc.vector.tensor_scalar(out=mu[:], in0=S[:],
                                        scalar1=1.0 / float(N_TOTAL),
                                        scalar2=None, op0=OP.mult)
                # x = mu - mu^2 + eps
                m2 = cp.tile([64, 1], F32)
                nc.vector.tensor_tensor(m2[:], mu[:], mu[:], OP.mult)
                x = cp.tile([64, 1], F32)
                nc.vector.tensor_tensor(x[:], mu[:], m2[:], OP.subtract)
                nc.vector.tensor_scalar(out=x[:], in0=x[:], scalar1=EPS,
                                        scalar2=None, op0=OP.add)
                # r ~= rsqrt(x) then 2 Newton iterations r *= 1.5 - 0.5*x*r^2
                sq = cp.tile([64, 1], F32)
                nc.scalar.activation(sq[:], x[:], AF.Sqrt)
                r = cp.tile([64, 1], F32)
                nc.vector.reciprocal(r[:], sq[:])
                for _ in range(2):
                    e = cp.tile([64, 1], F32)
                    nc.vector.tensor_tensor(e[:], r[:], r[:], OP.mult)
                    nc.vector.tensor_tensor(e[:], e[:], x[:], OP.mult)
                    nc.vector.tensor_scalar(out=e[:], in0=e[:], scalar1=-0.5,
                                            scalar2=1.5, op0=OP.mult,
                                            op1=OP.add)
                    nc.vector.tensor_tensor(r[:], r[:], e[:], OP.mult)
                # a = gamma*r ; scale = a/2 ; bias = a/2 + beta - mu*a
                a = cp.tile([64, 1], F32)
                nc.vector.tensor_tensor(a[:], gam_t[:], r[:], OP.mult)
                sc64 = cp.tile([64, 1], F32)
                nc.vector.tensor_scalar(out=sc64[:], in0=a[:], scalar1=0.5,
                                        scalar2=None, op0=OP.mult)
                tmp = cp.tile([64, 1], F32)
                nc.vector.tensor_tensor(tmp[:], mu[:], a[:], OP.mult)
                b0 = cp.tile([64, 1], F32)
                nc.vector.tensor_tensor(b0[:], bet_t[:], tmp[:], OP.subtract)
                bi64 = cp.tile([64, 1], F32)
                nc.vector.tensor_tensor(bi64[:], sc64[:], b0[:], OP.add)
                sc128 = cp.tile([128, 1], F32)
                bi128 = cp.tile([128, 1], F32)
                nc.sync.dma_start(sc128[0:64, :], sc64[:])
                nc.sync.dma_start(sc128[64:128, :], sc64[:])
                nc.sync.dma_start(bi128[0:64, :], bi64[:])
                nc.sync.dma_start(bi128[64:128, :], bi64[:])

                # ---------------- pass 2 ----------------
                for pair in range(NPAIR):
                    ot = op_.tile([128, T, BL], F16)
                    nc.scalar.activation(
                        ot[:], store[:, pair * T:(pair + 1) * T, :],
                        AF.Identity, bias=bi128[:], scale=sc128[:])
                    nc.sync.dma_start(out_r[2 * pair], ot[0:64])
                    nc.sync.dma_start(out_r[2 * pair + 1], ot[64:128])

    nc.compile()
    return nc


def _prep_host(dec, enc, Wg, bg, gamma, beta):
    Wg = np.asarray(Wg, dtype=np.float32)
    wdT = np.ascontiguousarray(Wg[:, :64].T)   # [k, m] dec-part
    weT = np.ascontiguousarray(Wg[:, 64:].T)   # enc-part
    wd = np.zeros((128, 128), dtype=np.float32)
    we = np.zeros((128, 128), dtype=np.float32)
    wd[:64, :64] = wdT
    wd[64:, 64:] = wdT
    we[:64, :64] = weT
    we[64:, 64:] = weT
    bgh = np.tile(0.5 * np.asarray(bg, np.float32), 2).reshape(128, 1)
    nth = np.full((128, 1), -TH, dtype=np.float32)
    idp = np.eye(128, dtype=np.float32) * 0.5
    idm = np.eye(128, dtype=np.float32) * -0.5
    gam = np.asarray(gamma, np.float32).reshape(64, 1)
    bet = np.asarray(beta, np.float32).reshape(64, 1)
    in_maps = []
    for b in range(NCORES):
        in_maps.append({
            "dec": np.ascontiguousarray(dec[:, b]).reshape(T, C, NPIX),
            "enc": np.ascontiguousarray(enc[:, b]).reshape(T, C, NPIX),
            "wd": wd, "we": we, "idp": idp, "idm": idm,
            "bgh": bgh, "nth": nth, "gam": gam, "bet": bet,
        })
    return in_maps


def kernel(dec, enc, Wg, bg, gamma, beta, _trace=False, _trace_kwargs=None):
    from concourse.bass_utils import run_bass_kernel_spmd

    if "nc" not in _cache:
        _cache["nc"] = _build()
    nc = _cache["nc"]

    in_maps = _prep_host(dec, enc, Wg, bg, gamma, beta)
    kw = {}
    if _trace:
        kw["trace"] = True
        if _trace_kwargs:
            kw.update(_trace_kwargs)
    res = run_bass_kernel_spmd(nc, in_maps, core_ids=list(range(NCORES)), **kw)
    out = np.stack([np.asarray(res.results[b]["out"]) for b in range(NCORES)],
                   axis=1)
    out = out.astype(np.float32).reshape(T, B, C, H, W)
    if _trace:
        _cache["last_res"] = res
    return out


# revision 23
# speedup vs baseline: 1.4518x; 1.3032x over previous
"""Trainium2 Bass kernel for GatedSkipFusion (gate conv -> sigmoid blend ->
4-step LIF -> BatchNorm with training stats).

Self-contained: hardcodes shapes T=4, B=8, C=64, H=W=112; shards batch B
across 8 NeuronCores; BN stats via a 64-float AllReduce.

Math:
  gate = sigmoid(pre); fused = enc + gate*(dec-enc). With h = tanh(pre/2):
  gate = 0.5 + 0.5*h, so fused = enc + 0.5*(1+h)*D, D = dec-enc.
  LIF (tau=2, hard reset, v_th=0.15): v_t = 0.5*v_{t-1}*m_{t-1} + fused_t,
  m = (v < th). Spikes are binary so BN var = mu - mu^2; the BN output is a
  per-channel affine of the sign record sg = Sign(v - th) in {-1,0,1}:
  out = (a/2)*sg + (a/2 + beta - mu*a), a = gamma*rsqrt(var+eps).

Engine split (all four compute engines + DMA overlap; the per-core program
is memory-bound at ~360 GB/s):
  PE    : gate matmuls in fp32r (1 cyc/row)
  Act   : batched tanh from a 4-bank PSUM tile; per-step Sign with
          accumulation for the BN statistics
  DVE   : gD=(1+h)*D, F=0.5*gD+enc, reset-mask mult, and the final
          affine as a 4x-mode fp16 tensor_scalar
  Pool  : D=dec-enc and the v-update scalar_tensor_tensor
  fp16 output (halves the output DMA; ~6e-4 systematic error).
"""

import numpy as np

T, B, C, H, W = 4, 8, 64, 112, 112
NPIX = H * W          # 12544
BL = 448              # pixel block (free dim)
NPAIR = NPIX // (2 * BL)   # 14 pairs of blocks
NTILE = NPAIR * T     # 56 (pair,t) tiles
TH = 0.15
EPS = 1e-5
NCORES = 8
N_TOTAL = T * B * NPIX     # 401408 per-channel element count
N_CORE = T * NPIX          # 50176 per-core per-channel count

_cache = {}


def _build(reps=1, use_collective=True, num_devices=NCORES, d_on_pe=True,
           skip=frozenset()):
    import concourse.bass as bass
    import concourse.bacc as bacc
    import concourse.mybir as mybir
    import concourse.tile as tile

    F32 = mybir.dt.float32
    F32R = mybir.dt.float32r
    F16 = mybir.dt.float16
    AF = mybir.ActivationFunctionType
    OP = mybir.AluOpType
    AX = mybir.AxisListType

    nc = bacc.Bacc("TRN2", target_bir_lowering=False, debug=False,
                   enable_asserts=False, num_devices=num_devices)

    # host pre-arranged layout: [pair, partition(p2*64+c), t, x]
    dec_d = nc.dram_tensor("dec", [NPAIR, 128, T, BL], F32R,
                           kind="ExternalInput")
    enc_d = nc.dram_tensor("enc", [NPAIR, 128, T, BL], F32R,
                           kind="ExternalInput")
    wd_d = nc.dram_tensor("wd", [128, 128], F32R, kind="ExternalInput")
    we_d = nc.dram_tensor("we", [128, 128], F32R, kind="ExternalInput")
    idp_d = nc.dram_tensor("idp", [128, 128], F32R, kind="ExternalInput")
    idm_d = nc.dram_tensor("idm", [128, 128], F32R, kind="ExternalInput")
    bgh_d = nc.dram_tensor("bgh", [128, 1], F32, kind="ExternalInput")
    nth_d = nc.dram_tensor("nth", [128, 1], F32, kind="ExternalInput")
    gam_d = nc.dram_tensor("gam", [64, 1], F32, kind="ExternalInput")
    bet_d = nc.dram_tensor("bet", [64, 1], F32, kind="ExternalInput")
    out_d = nc.dram_tensor("out", [NPAIR, 128, T, BL], F16,
                           kind="ExternalOutput")

    with tile.TileContext(nc) as tc:
        with tc.tile_pool(name="const", bufs=1) as cp, \
             tc.tile_pool(name="io", bufs=3) as io, \
             tc.tile_pool(name="wk", bufs=3) as wk, \
             tc.tile_pool(name="wkg", bufs=2) as wkg, \
             tc.tile_pool(name="wkf", bufs=4) as wkf, \
             tc.tile_pool(name="vv", bufs=4) as vv, \
             tc.tile_pool(name="sm", bufs=6) as sm, \
             tc.tile_pool(name="ot", bufs=2) as op_, \
             tc.tile_pool(name="ps", bufs=1, space="PSUM") as ps, \
             tc.tile_pool(name="psd", bufs=1, space="PSUM") as psd, \
             tc.tile_pool(name="dram", bufs=2, space="DRAM") as dp:

            wd_t = cp.tile([128, 128], F32R)
            we_t = cp.tile([128, 128], F32R)
            idp_t = cp.tile([128, 128], F32R)
            idm_t = cp.tile([128, 128], F32R)
            nc.sync.dma_start(idp_t[:], idp_d[:, :])
            nc.sync.dma_start(idm_t[:], idm_d[:, :])
            bgh_t = cp.tile([128, 1], F32)
            nth_t = cp.tile([128, 1], F32)
            gam_t = cp.tile([64, 1], F32)
            bet_t = cp.tile([64, 1], F32)
            nc.sync.dma_start(wd_t[:], wd_d[:, :])
            nc.sync.dma_start(we_t[:], we_d[:, :])
            nc.sync.dma_start(bgh_t[:], bgh_d[:, :])
            nc.sync.dma_start(nth_t[:], nth_d[:, :])
            nc.sync.dma_start(gam_t[:], gam_d[:, :])
            nc.sync.dma_start(bet_t[:], bet_d[:, :])

            store = cp.tile([128, NTILE, BL], F16)    # sign record
            scol = cp.tile([128, NTILE], F32)         # per-tile sign sums

            for _rep in range(reps):
                # ---------------- pass 1 (software-pipelined) ----------------
                # Stage lags per emission iteration k:
                #   dma(k); pre/h/D(k-1); gD/F(k-2); lif(k-3); signs(k-4/k-3)
                # Every instruction's inputs were produced in an earlier
                # iteration, so no engine stream ever head-of-line blocks.
                dec4s, enc4s = {}, {}
                h4s, gD4s, F4s = {}, {}, {}
                vps = {}      # pair -> list of v tiles (per t)

                def emit_dma(p):
                    dec4 = io.tile([128, T, BL], F32R)
                    enc4 = io.tile([128, T, BL], F32R)
                    nc.sync.dma_start(dec4[:], dec_d[p])
                    nc.sync.dma_start(enc4[:], enc_d[p])
                    dec4s[p], enc4s[p] = dec4, enc4

                def emit_signs(p):
                    # sign for (p, 1..3) plus (p+1, 0): all deps one iter old
                    for t in range(1, T):
                        if 0 <= p < NPAIR:
                            idx = p * T + t
                            nc.scalar.activation(
                                store[:, idx], vps[p][t], AF.Sign,
                                bias=nth_t[:], scale=1.0,
                                accum_out=scol[:, idx:idx + 1])
                    q = p + 1
                    if 0 <= q < NPAIR:
                        idx = q * T
                        nc.scalar.activation(
                            store[:, idx], F4s[q][:, 0], AF.Sign,
                            bias=nth_t[:], scale=1.0,
                            accum_out=scol[:, idx:idx + 1])

                def emit_pre_h(p):
                    dec4, enc4 = dec4s[p], enc4s[p]
                    P4 = ps.tile([128, T, 512], F32)
                    for t in range(T):
                        nc.tensor.matmul(out=P4[:, t, 0:BL], lhsT=wd_t[:],
                                         rhs=dec4[:, t], start=True,
                                         stop=False)
                        nc.tensor.matmul(out=P4[:, t, 0:BL], lhsT=we_t[:],
                                         rhs=enc4[:, t], start=False,
                                         stop=True)
                    h4 = wk.tile([128, T, BL], F32)
                    nc.scalar.activation(h4[:], P4[:, :, 0:BL], AF.Tanh,
                                         bias=bgh_t[:], scale=0.5)
                    h4s[p] = h4

                def emit_D(p):
                    dec4, enc4 = dec4s[p], enc4s[p]
                    D4ap = psd.tile([128, T, 512], F32)
                    for t in range(T):
                        nc.tensor.matmul(out=D4ap[:, t, 0:BL],
                                         lhsT=idp_t[:], rhs=dec4[:, t],
                                         start=True, stop=False)
                        nc.tensor.matmul(out=D4ap[:, t, 0:BL],
                                         lhsT=idm_t[:], rhs=enc4[:, t],
                                         start=False, stop=True)
                    return D4ap

                def emit_gD_F(p, D4ap):
                    # gD = (1+h)*0.5*(dec-enc) = sigma(pre)*(dec-enc)
                    gD4 = wkg.tile([128, T, BL], F32)
                    nc.vector.scalar_tensor_tensor(out=gD4[:], in0=h4s[p][:],
                                                   scalar=1.0,
                                                   in1=D4ap[:, :, 0:BL],
                                                   op0=OP.add, op1=OP.mult)
                    F4 = wkf.tile([128, T, BL], F32)
                    nc.vector.scalar_tensor_tensor(out=F4[:], in0=gD4[:],
                                                   scalar=1.0,
                                                   in1=enc4s[p][:].bitcast(F32),
                                                   op0=OP.mult, op1=OP.add)
                    F4s[p] = F4
                    del h4s[p]
                    del dec4s[p], enc4s[p]

                def emit_lif(p):
                    F4 = F4s[p]
                    vp = F4[:, 0]
                    vlist = [vp]
                    for t in range(T - 1):
                        vrn = vv.tile([128, BL], F32)
                        nc.vector.scalar_tensor_tensor(
                            out=vrn[:], in0=vp, scalar=TH, in1=vp,
                            op0=OP.is_lt, op1=OP.mult)
                        vpt = sm.tile([128, BL], F32)
                        nc.vector.scalar_tensor_tensor(
                            out=vpt[:], in0=vrn[:], scalar=0.5,
                            in1=F4[:, t + 1], op0=OP.mult, op1=OP.add)
                        vp = vpt[:]
                        vlist.append(vp)
                    vps[p] = vlist

                D4prev = {}
                for k in range(NPAIR + 4):
                    if k >= 3:
                        emit_signs(k - 4)   # signs for pair k-4 (t>=1)
                                            # and pair k-3 (t=0)
                    if k < NPAIR:
                        emit_dma(k)
                    if 0 <= k - 1 < NPAIR:
                        emit_pre_h(k - 1)
                        D4prev[k - 1] = emit_D(k - 1)
                    if 0 <= k - 2 < NPAIR:
                        emit_gD_F(k - 2, D4prev.pop(k - 2))
                    if 0 <= k - 3 < NPAIR:
                        emit_lif(k - 3)

                # ---------------- stats ----------------
                red = cp.tile([128, 1], F32)
                nc.vector.tensor_reduce(out=red[:], in_=scol[:, 0:NTILE],
                                        axis=AX.X, op=OP.add)
                redB = cp.tile([64, 1], F32)
                nc.sync.dma_start(redB[:], red[64:128, :])
                s64 = cp.tile([64, 1], F32)
                nc.vector.tensor_tensor(s64[:], red[0:64, :], redB[:], OP.add)
                # local spike count = 0.5*sum_sign + N_CORE/2
                loc = cp.tile([64, 1], F32)
                nc.vector.tensor_scalar(out=loc[:], in0=s64[:], scalar1=0.5,
                                        scalar2=float(N_CORE) / 2.0,
                                        op0=OP.mult, op1=OP.add)
                cin = dp.tile([64, 1], F32)
                cout = dp.tile([64, 1], F32)
                S = cp.tile([64, 1], F32)
                if use_collective:
                    nc.sync.dma_start(cin[:], loc[:])
                    nc.gpsimd.collective_compute(
                        "AllReduce", OP.add,
                        replica_groups=[list(range(num_devices))],
                        ins=[cin.opt()], outs=[cout.opt()])
                    nc.sync.dma_start(S[:], cout[:])
                else:
                    nc.vector.tensor_scalar(out=S[:], in0=loc[:],
                                            scalar1=float(NCORES),
                                            scalar2=None, op0=OP.mult)

                mu = cp.tile([64, 1], F32)
                nc.vector.tensor_scalar(out=mu[:], in0=S[:],
                                        scalar1=1.0 / float(N_TOTAL),
                                        scalar2=None, op0=OP.mult)
                # x = mu - mu^2 + eps
                m2 = cp.tile([64, 1], F32)
                nc.vector.tensor_tensor(m2[:], mu[:], mu[:], OP.mult)
                x = cp.tile([64, 1], F32)
                nc.vector.tensor_tensor(x[:], mu[:], m2[:], OP.subtract)
                nc.vector.tensor_scalar(out=x[:], in0=x[:], scalar1=EPS,
                                        scalar2=None, op0=OP.add)
                # r ~= rsqrt(x) then 2 Newton iterations r *= 1.5 - 0.5*x*r^2
                sq = cp.tile([64, 1], F32)
                nc.scalar.activation(sq[:], x[:], AF.Sqrt)
                r = cp.tile([64, 1], F32)
                nc.vector.reciprocal(r[:], sq[:])
                for _ in range(2):
                    e = cp.tile([64, 1], F32)
                    nc.vector.tensor_tensor(e[:], r[:], r[:], OP.mult)
                    nc.vector.tensor_tensor(e[:], e[:], x[:], OP.mult)
                    nc.vector.tensor_scalar(out=e[:], in0=e[:], scalar1=-0.5,
                                            scalar2=1.5, op0=OP.mult,
                                            op1=OP.add)
                    nc.vector.tensor_tensor(r[:], r[:], e[:], OP.mult)
                # a = gamma*r ; scale = a/2 ; bias = a/2 + beta - mu*a
                a = cp.tile([64, 1], F32)
                nc.vector.tensor_tensor(a[:], gam_t[:], r[:], OP.mult)
                sc64 = cp.tile([64, 1], F32)
                nc.vector.tensor_scalar(out=sc64[:], in0=a[:], scalar1=0.5,
                                        scalar2=None, op0=OP.mult)
                tmp = cp.tile([64, 1], F32)
                nc.vector.tensor_tensor(tmp[:], mu[:], a[:], OP.mult)
                b0 = cp.tile([64, 1], F32)
                nc.vector.tensor_tensor(b0[:], bet_t[:], tmp[:], OP.subtract)
                bi64 = cp.tile([64, 1], F32)
                nc.vector.tensor_tensor(bi64[:], sc64[:], b0[:], OP.add)
                sc128 = cp.tile([128, 1], F32)
                bi128 = cp.tile([128, 1], F32)
                nc.sync.dma_start(sc128[0:64, :], sc64[:])
                nc.sync.dma_start(sc128[64:128, :], sc64[:])
                nc.sync.dma_start(bi128[0:64, :], bi64[:])
                nc.sync.dma_start(bi128[64:128, :], bi64[:])

                # ---------------- pass 2 ----------------
                for pair in range(NPAIR):
                    ot = op_.tile([128, T, BL], F16)
                    nc.vector.tensor_scalar(
                        out=ot[:], in0=store[:, pair * T:(pair + 1) * T, :],
                        scalar1=sc128[:], scalar2=bi128[:],
                        op0=OP.mult, op1=OP.add)
                    nc.sync.dma_start(out_d[pair], ot[:])

    nc.compile()
    return nc


def _prep_host(dec, enc, Wg, bg, gamma, beta):
    Wg = np.asarray(Wg, dtype=np.float32)
    wdT = np.ascontiguousarray(Wg[:, :64].T)   # [k, m] dec-part
    weT = np.ascontiguousarray(Wg[:, 64:].T)   # enc-part
    wd = np.zeros((128, 128), dtype=np.float32)
    we = np.zeros((128, 128), dtype=np.float32)
    wd[:64, :64] = wdT
    wd[64:, 64:] = wdT
    we[:64, :64] = weT
    we[64:, 64:] = weT
    bgh = np.tile(0.5 * np.asarray(bg, np.float32), 2).reshape(128, 1)
    nth = np.full((128, 1), -TH, dtype=np.float32)
    idp = np.eye(128, dtype=np.float32) * 0.5
    idm = np.eye(128, dtype=np.float32) * -0.5

    def relayout(x):
        # [T, C, NPIX] -> [pair, p2*64+c, t, x448]
        x = np.asarray(x, np.float32).reshape(T, C, NPAIR, 2, BL)
        return np.ascontiguousarray(x.transpose(2, 3, 1, 0, 4)
                                    .reshape(NPAIR, 128, T, BL))
    gam = np.asarray(gamma, np.float32).reshape(64, 1)
    bet = np.asarray(beta, np.float32).reshape(64, 1)
    in_maps = []
    for b in range(NCORES):
        in_maps.append({
            "dec": relayout(np.asarray(dec[:, b]).reshape(T, C, NPIX)),
            "enc": relayout(np.asarray(enc[:, b]).reshape(T, C, NPIX)),
            "wd": wd, "we": we, "idp": idp, "idm": idm,
            "bgh": bgh, "nth": nth, "gam": gam, "bet": bet,
        })
    return in_maps


def kernel(dec, enc, Wg, bg, gamma, beta, _trace=False, _trace_kwargs=None):
    from concourse.bass_utils import run_bass_kernel_spmd

    if "nc" not in _cache:
        _cache["nc"] = _build()
    nc = _cache["nc"]

    in_maps = _prep_host(dec, enc, Wg, bg, gamma, beta)
    kw = {}
    if _trace:
        kw["trace"] = True
        if _trace_kwargs:
            kw.update(_trace_kwargs)
    res = run_bass_kernel_spmd(nc, in_maps, core_ids=list(range(NCORES)), **kw)
    outs = []
    for b in range(NCORES):
        o = np.asarray(res.results[b]["out"]).astype(np.float32)
        # [pair, p2*64+c, t, x448] -> [T, C, NPIX]
        o = o.reshape(NPAIR, 2, C, T, BL).transpose(3, 2, 0, 1, 4)
        outs.append(o.reshape(T, C, NPIX))
    out = np.stack(outs, axis=1).reshape(T, B, C, H, W)
    if _trace:
        _cache["last_res"] = res
    return out


# revision 35
# speedup vs baseline: 1.5693x; 1.0809x over previous
"""Trainium2 Bass kernel for GatedSkipFusion (gate conv -> sigmoid blend ->
4-step LIF -> BatchNorm with training stats).

Self-contained: hardcodes shapes T=4, B=8, C=64, H=W=112; shards batch B
across 8 NeuronCores; BN stats via a 64-float AllReduce.

Math:
  gate = sigmoid(pre); fused = enc + gate*(dec-enc). With h = tanh(pre/2):
  gate = 0.5 + 0.5*h, so fused = enc + 0.5*(1+h)*D, D = dec-enc.
  LIF (tau=2, hard reset, v_th=0.15): v_t = 0.5*v_{t-1}*m_{t-1} + fused_t,
  m = (v < th). Spikes are binary so BN var = mu - mu^2; the BN output is a
  per-channel affine of the sign record sg = Sign(v - th) in {-1,0,1}:
  out = (a/2)*sg + (a/2 + beta - mu*a), a = gamma*rsqrt(var+eps).

Engine split (all four compute engines + DMA overlap; the per-core program
is memory-bound at ~360 GB/s):
  PE    : gate matmuls in fp32r (1 cyc/row)
  Act   : batched tanh from a 4-bank PSUM tile; per-step Sign with
          accumulation for the BN statistics
  DVE   : gD=(1+h)*D, F=0.5*gD+enc, reset-mask mult, and the final
          affine as a 4x-mode fp16 tensor_scalar
  Pool  : D=dec-enc and the v-update scalar_tensor_tensor
  fp16 output (halves the output DMA; ~6e-4 systematic error).
"""

import numpy as np

T, B, C, H, W = 4, 8, 64, 112, 112
NPIX = H * W          # 12544
BL = 448              # pixel block (free dim)
NPAIR = NPIX // (2 * BL)   # 14 pairs of blocks
NTILE = NPAIR * T     # 56 (pair,t) tiles
TH = 0.15
EPS = 1e-5
NCORES = 8
N_TOTAL = T * B * NPIX     # 401408 per-channel element count
N_CORE = T * NPIX          # 50176 per-core per-channel count

_cache = {}


def _build(reps=1, use_collective=True, num_devices=NCORES, d_on_pe=True,
           skip=frozenset()):
    import concourse.bass as bass
    import concourse.bacc as bacc
    import concourse.mybir as mybir
    import concourse.tile as tile

    F32 = mybir.dt.float32
    F32R = mybir.dt.float32r
    F16 = mybir.dt.float16
    AF = mybir.ActivationFunctionType
    OP = mybir.AluOpType
    AX = mybir.AxisListType

    nc = bacc.Bacc("TRN2", target_bir_lowering=False, debug=False,
                   enable_asserts=False, num_devices=num_devices)

    # host pre-arranged layout: [pair, partition(p2*64+c), t, x]
    dec_d = nc.dram_tensor("dec", [NPAIR, 128, T, BL], F32R,
                           kind="ExternalInput")
    enc_d = nc.dram_tensor("enc", [NPAIR, 128, T, BL], F32R,
                           kind="ExternalInput")
    wd_d = nc.dram_tensor("wd", [128, 128], F32R, kind="ExternalInput")
    we_d = nc.dram_tensor("we", [128, 128], F32R, kind="ExternalInput")
    idp_d = nc.dram_tensor("idp", [128, 128], F32R, kind="ExternalInput")
    idm_d = nc.dram_tensor("idm", [128, 128], F32R, kind="ExternalInput")
    i2_d = nc.dram_tensor("i2", [128, 64], F32R, kind="ExternalInput")
    bgh_d = nc.dram_tensor("bgh", [128, 1], F32, kind="ExternalInput")
    nth_d = nc.dram_tensor("nth", [128, 1], F32, kind="ExternalInput")
    gam_d = nc.dram_tensor("gam", [128, 1], F32, kind="ExternalInput")
    bet_d = nc.dram_tensor("bet", [128, 1], F32, kind="ExternalInput")
    out_d = nc.dram_tensor("out", [NPAIR, 128, T, BL], F16,
                           kind="ExternalOutput")

    with tile.TileContext(nc) as tc:
        with tc.tile_pool(name="const", bufs=1) as cp, \
             tc.tile_pool(name="io", bufs=3) as io, \
             tc.tile_pool(name="wk", bufs=3) as wk, \
             tc.tile_pool(name="wkg", bufs=2) as wkg, \
             tc.tile_pool(name="wkf", bufs=3) as wkf, \
             tc.tile_pool(name="vv", bufs=4) as vv, \
             tc.tile_pool(name="sm", bufs=6) as sm, \
             tc.tile_pool(name="ot", bufs=6) as op_, \
             tc.tile_pool(name="ps", bufs=1, space="PSUM") as ps, \
             tc.tile_pool(name="psd", bufs=1, space="PSUM") as psd, \
             tc.tile_pool(name="dram", bufs=2, space="DRAM") as dp:

            wd_t = cp.tile([128, 128], F32R)
            we_t = cp.tile([128, 128], F32R)
            idp_t = cp.tile([128, 128], F32R)
            idm_t = cp.tile([128, 128], F32R)
            i2_t = cp.tile([128, 64], F32R)
            nc.sync.dma_start(idp_t[:], idp_d[:, :])
            nc.sync.dma_start(idm_t[:], idm_d[:, :])
            nc.sync.dma_start(i2_t[:], i2_d[:, :])
            bgh_t = cp.tile([128, 1], F32)
            nth_t = cp.tile([128, 1], F32)
            gam_t = cp.tile([128, 1], F32)
            bet_t = cp.tile([128, 1], F32)
            nc.sync.dma_start(wd_t[:], wd_d[:, :])
            nc.sync.dma_start(we_t[:], we_d[:, :])
            nc.sync.dma_start(bgh_t[:], bgh_d[:, :])
            nc.sync.dma_start(nth_t[:], nth_d[:, :])
            nc.sync.dma_start(gam_t[:], gam_d[:, :])
            nc.sync.dma_start(bet_t[:], bet_d[:, :])

            store = cp.tile([128, NTILE, BL], F16)    # sign record
            scol = cp.tile([128, NTILE], F32)         # per-tile sign sums

            for _rep in range(reps):
                # ---------------- pass 1 (software-pipelined) ----------------
                # Stage lags per emission iteration k:
                #   dma(k); pre/h/D(k-1); gD/F(k-2); lif(k-3); signs(k-4/k-3)
                # Every instruction's inputs were produced in an earlier
                # iteration, so no engine stream ever head-of-line blocks.
                dec4s, enc4s = {}, {}
                h4s, gD4s, F4s = {}, {}, {}
                vps = {}      # pair -> list of v tiles (per t)

                def emit_dma(p):
                    # halves so the first matmuls unblock after 1/4 of the
                    # pair's bytes (shortens pipeline fill)
                    dec4 = io.tile([128, T, BL], F32R)
                    enc4 = io.tile([128, T, BL], F32R)
                    nc.sync.dma_start(dec4[:, 0:2], dec_d[p, :, 0:2])
                    nc.sync.dma_start(enc4[:, 0:2], enc_d[p, :, 0:2])
                    nc.sync.dma_start(dec4[:, 2:4], dec_d[p, :, 2:4])
                    nc.sync.dma_start(enc4[:, 2:4], enc_d[p, :, 2:4])
                    dec4s[p], enc4s[p] = dec4, enc4

                def emit_signs(p):
                    # sign for (p, 1..3) plus (p+1, 0): all deps one iter old
                    for t in range(1, T):
                        if 0 <= p < NPAIR:
                            idx = p * T + t
                            nc.scalar.activation(
                                store[:, idx], vps[p][t], AF.Sign,
                                bias=nth_t[:], scale=1.0,
                                accum_out=scol[:, idx:idx + 1])
                    q = p + 1
                    if 0 <= q < NPAIR:
                        idx = q * T
                        nc.scalar.activation(
                            store[:, idx], F4s[q][:, 0], AF.Sign,
                            bias=nth_t[:], scale=1.0,
                            accum_out=scol[:, idx:idx + 1])

                def emit_pre_h(p):
                    dec4, enc4 = dec4s[p], enc4s[p]
                    P4 = ps.tile([128, T, 512], F32)
                    for t in range(T):
                        nc.tensor.matmul(out=P4[:, t, 0:BL], lhsT=wd_t[:],
                                         rhs=dec4[:, t], start=True,
                                         stop=False)
                        nc.tensor.matmul(out=P4[:, t, 0:BL], lhsT=we_t[:],
                                         rhs=enc4[:, t], start=False,
                                         stop=True)
                    h4 = wk.tile([128, T, BL], F32)
                    nc.scalar.activation(h4[:], P4[:, :, 0:BL], AF.Tanh,
                                         bias=bgh_t[:], scale=0.5)
                    h4s[p] = h4

                def emit_D(p):
                    dec4, enc4 = dec4s[p], enc4s[p]
                    D4ap = psd.tile([128, T, 512], F32)
                    for t in range(T):
                        nc.tensor.matmul(out=D4ap[:, t, 0:BL],
                                         lhsT=idp_t[:], rhs=dec4[:, t],
                                         start=True, stop=False)
                        nc.tensor.matmul(out=D4ap[:, t, 0:BL],
                                         lhsT=idm_t[:], rhs=enc4[:, t],
                                         start=False, stop=True)
                    return D4ap

                def emit_gD_F(p, D4ap):
                    # gD = (1+h)*0.5*(dec-enc) = sigma(pre)*(dec-enc)
                    gD4 = wkg.tile([128, T, BL], F32)
                    nc.vector.scalar_tensor_tensor(out=gD4[:], in0=h4s[p][:],
                                                   scalar=1.0,
                                                   in1=D4ap[:, :, 0:BL],
                                                   op0=OP.add, op1=OP.mult)
                    F4 = wkf.tile([128, T, BL], F32)
                    nc.vector.scalar_tensor_tensor(out=F4[:], in0=gD4[:],
                                                   scalar=1.0,
                                                   in1=enc4s[p][:].bitcast(F32),
                                                   op0=OP.mult, op1=OP.add)
                    F4s[p] = F4
                    del h4s[p]
                    del dec4s[p], enc4s[p]

                def emit_lif(p):
                    F4 = F4s[p]
                    vp = F4[:, 0]
                    vlist = [vp]
                    for t in range(T - 1):
                        vrn = vv.tile([128, BL], F32)
                        nc.vector.scalar_tensor_tensor(
                            out=vrn[:], in0=vp, scalar=TH, in1=vp,
                            op0=OP.is_lt, op1=OP.mult)
                        vpt = sm.tile([128, BL], F32)
                        nc.vector.scalar_tensor_tensor(
                            out=vpt[:], in0=vrn[:], scalar=0.5,
                            in1=F4[:, t + 1], op0=OP.mult, op1=OP.add)
                        vp = vpt[:]
                        vlist.append(vp)
                    vps[p] = vlist

                D4prev = {}
                for k in range(NPAIR + 4):
                    if k >= 3:
                        emit_signs(k - 4)   # signs for pair k-4 (t>=1)
                                            # and pair k-3 (t=0)
                    if k < NPAIR:
                        emit_dma(k)
                    if 0 <= k - 1 < NPAIR:
                        emit_pre_h(k - 1)
                        D4prev[k - 1] = emit_D(k - 1)
                    if 0 <= k - 2 < NPAIR:
                        emit_gD_F(k - 2, D4prev.pop(k - 2))
                    if 0 <= k - 3 < NPAIR:
                        emit_lif(k - 3)

                # ---------------- stats ----------------
                # per-channel sign sum: contract the two 64-partition halves
                # of scol with a stacked identity on PE, then reduce tiles
                red = cp.tile([128, 1], F32)
                nc.vector.tensor_reduce(out=red[:], in_=scol[:, 0:NTILE],
                                        axis=AX.X, op=OP.add)
                redB = cp.tile([64, 1], F32)
                nc.sync.dma_start(redB[:], red[64:128, :])
                s64 = cp.tile([64, 1], F32)
                nc.vector.tensor_tensor(s64[:], red[0:64, :], redB[:], OP.add)
                # local spike count = 0.5*sum_sign + N_CORE/2
                loc = cp.tile([64, 1], F32)
                nc.vector.tensor_scalar(out=loc[:], in0=s64[:], scalar1=0.5,
                                        scalar2=float(N_CORE) / 2.0,
                                        op0=OP.mult, op1=OP.add)
                S128 = cp.tile([128, 1], F32)
                if use_collective:
                    cin = dp.tile([64, 1], F32)
                    cout = dp.tile([64, 1], F32)
                    nc.sync.dma_start(cin[:], loc[:])
                    nc.gpsimd.collective_compute(
                        "AllReduce", OP.add,
                        replica_groups=[list(range(num_devices))],
                        ins=[cin.opt()], outs=[cout.opt()])
                    nc.sync.dma_start(S128[0:64, :], cout[:])
                    nc.sync.dma_start(S128[64:128, :], cout[:])
                else:
                    Sl = cp.tile([64, 1], F32)
                    nc.vector.tensor_scalar(out=Sl[:], in0=loc[:],
                                            scalar1=float(NCORES),
                                            scalar2=None, op0=OP.mult)
                    nc.sync.dma_start(S128[0:64, :], Sl[:])
                    nc.sync.dma_start(S128[64:128, :], Sl[:])

                # all remaining stats math on 128 partitions directly
                mu = cp.tile([128, 1], F32)
                nc.vector.tensor_scalar(out=mu[:], in0=S128[:],
                                        scalar1=1.0 / float(N_TOTAL),
                                        scalar2=None, op0=OP.mult)
                # x = mu*(1-mu) + eps
                m1 = cp.tile([128, 1], F32)
                nc.vector.tensor_scalar(out=m1[:], in0=mu[:], scalar1=-1.0,
                                        scalar2=1.0, op0=OP.mult, op1=OP.add)
                x = cp.tile([128, 1], F32)
                nc.vector.tensor_tensor(x[:], m1[:], mu[:], OP.mult)
                nc.vector.tensor_scalar(out=x[:], in0=x[:], scalar1=EPS,
                                        scalar2=None, op0=OP.add)
                # r = 1/sqrt(x) + one Newton step r *= 1.5-0.5*x*r^2
                sq = cp.tile([128, 1], F32)
                nc.scalar.activation(sq[:], x[:], AF.Sqrt)
                r0 = cp.tile([128, 1], F32)
                nc.vector.reciprocal(r0[:], sq[:])
                e = cp.tile([128, 1], F32)
                nc.vector.tensor_tensor(e[:], r0[:], r0[:], OP.mult)
                nc.vector.tensor_tensor(e[:], e[:], x[:], OP.mult)
                nc.vector.tensor_scalar(out=e[:], in0=e[:], scalar1=-0.5,
                                        scalar2=1.5, op0=OP.mult, op1=OP.add)
                r = cp.tile([128, 1], F32)
                nc.vector.tensor_tensor(r[:], r0[:], e[:], OP.mult)
                # a = gamma*r ; scale = a/2 ; bias = a/2 + beta - mu*a
                a = cp.tile([128, 1], F32)
                nc.vector.tensor_tensor(a[:], gam_t[:], r[:], OP.mult)
                sc128 = cp.tile([128, 1], F32)
                nc.vector.tensor_scalar(out=sc128[:], in0=a[:], scalar1=0.5,
                                        scalar2=None, op0=OP.mult)
                tmp = cp.tile([128, 1], F32)
                nc.vector.tensor_tensor(tmp[:], mu[:], a[:], OP.mult)
                b0 = cp.tile([128, 1], F32)
                nc.vector.tensor_tensor(b0[:], bet_t[:], tmp[:], OP.subtract)
                bi128 = cp.tile([128, 1], F32)
                nc.vector.tensor_tensor(bi128[:], sc128[:], b0[:], OP.add)

                # ---------------- pass 2 ----------------
                for pair in range(NPAIR):
                    ot = op_.tile([128, T, BL], F16)
                    nc.vector.tensor_scalar(
                        out=ot[:], in0=store[:, pair * T:(pair + 1) * T, :],
                        scalar1=sc128[:], scalar2=bi128[:],
                        op0=OP.mult, op1=OP.add)
                    nc.sync.dma_start(out_d[pair], ot[:])

    nc.compile()
    return nc


def _prep_host(dec, enc, Wg, bg, gamma, beta):
    Wg = np.asarray(Wg, dtype=np.float32)
    wdT = np.ascontiguousarray(Wg[:, :64].T)   # [k, m] dec-part
    weT = np.ascontiguousarray(Wg[:, 64:].T)   # enc-part
    wd = np.zeros((128, 128), dtype=np.float32)
    we = np.zeros((128, 128), dtype=np.float32)
    wd[:64, :64] = wdT
    wd[64:, 64:] = wdT
    we[:64, :64] = weT
    we[64:, 64:] = weT
    bgh = np.tile(0.5 * np.asarray(bg, np.float32), 2).reshape(128, 1)
    nth = np.full((128, 1), -TH, dtype=np.float32)
    idp = np.eye(128, dtype=np.float32) * 0.5
    idm = np.eye(128, dtype=np.float32) * -0.5

    def relayout(x):
        # [T, C, NPIX] -> [pair, p2*64+c, t, x448]
        x = np.asarray(x, np.float32).reshape(T, C, NPAIR, 2, BL)
        return np.ascontiguousarray(x.transpose(2, 3, 1, 0, 4)
                                    .reshape(NPAIR, 128, T, BL))
    gam = np.tile(np.asarray(gamma, np.float32), 2).reshape(128, 1)
    bet = np.tile(np.asarray(beta, np.float32), 2).reshape(128, 1)
    i2 = np.ascontiguousarray(np.tile(np.eye(64, dtype=np.float32), (2, 1)))
    in_maps = []
    for b in range(NCORES):
        in_maps.append({
            "dec": relayout(np.asarray(dec[:, b]).reshape(T, C, NPIX)),
            "enc": relayout(np.asarray(enc[:, b]).reshape(T, C, NPIX)),
            "wd": wd, "we": we, "idp": idp, "idm": idm, "i2": i2,
            "bgh": bgh, "nth": nth, "gam": gam, "bet": bet,
        })
    return in_maps


def kernel(dec, enc, Wg, bg, gamma, beta, _trace=False, _trace_kwargs=None):
    from concourse.bass_utils import run_bass_kernel_spmd

    if "nc" not in _cache:
        _cache["nc"] = _build()
    nc = _cache["nc"]

    in_maps = _prep_host(dec, enc, Wg, bg, gamma, beta)
    kw = {}
    if _trace:
        kw["trace"] = True
        if _trace_kwargs:
            kw.update(_trace_kwargs)
    res = run_bass_kernel_spmd(nc, in_maps, core_ids=list(range(NCORES)), **kw)
    outs = []
    for b in range(NCORES):
        o = np.asarray(res.results[b]["out"]).astype(np.float32)
        # [pair, p2*64+c, t, x448] -> [T, C, NPIX]
        o = o.reshape(NPAIR, 2, C, T, BL).transpose(3, 2, 0, 1, 4)
        outs.append(o.reshape(T, C, NPIX))
    out = np.stack(outs, axis=1).reshape(T, B, C, H, W)
    if _trace:
        _cache["last_res"] = res
    return out


# revision 44
# speedup vs baseline: 1.6015x; 1.0205x over previous
"""Trainium2 Bass kernel for GatedSkipFusion (gate conv -> sigmoid blend ->
4-step LIF -> BatchNorm with training stats).

Self-contained: hardcodes shapes T=4, B=8, C=64, H=W=112; shards batch B
across 8 NeuronCores; BN stats via a 64-float AllReduce.

Math:
  gate = sigmoid(pre); fused = enc + gate*(dec-enc). With h = tanh(pre/2):
  gate = 0.5 + 0.5*h, so fused = enc + 0.5*(1+h)*D, D = dec-enc.
  LIF (tau=2, hard reset, v_th=0.15): v_t = 0.5*v_{t-1}*m_{t-1} + fused_t,
  m = (v < th). Spikes are binary so BN var = mu - mu^2; the BN output is a
  per-channel affine of the sign record sg = Sign(v - th) in {-1,0,1}:
  out = (a/2)*sg + (a/2 + beta - mu*a), a = gamma*rsqrt(var+eps).

Engine split (all four compute engines + DMA overlap; the per-core program
is memory-bound at ~360 GB/s):
  PE    : gate matmuls in fp32r (1 cyc/row)
  Act   : batched tanh from a 4-bank PSUM tile; per-step Sign with
          accumulation for the BN statistics
  DVE   : gD=(1+h)*D, F=0.5*gD+enc, reset-mask mult, and the final
          affine as a 4x-mode fp16 tensor_scalar
  Pool  : D=dec-enc and the v-update scalar_tensor_tensor
  fp16 output (halves the output DMA; ~6e-4 systematic error).
"""

import numpy as np

T, B, C, H, W = 4, 8, 64, 112, 112
NPIX = H * W          # 12544
BL = 448              # pixel block (free dim)
NPAIR = NPIX // (2 * BL)   # 14 pairs of blocks
NTILE = NPAIR * T     # 56 (pair,t) tiles
TH = 0.15
EPS = 1e-5
NCORES = 8
N_TOTAL = T * B * NPIX     # 401408 per-channel element count
N_CORE = T * NPIX          # 50176 per-core per-channel count

_cache = {}


def _build(reps=1, use_collective=True, num_devices=NCORES, d_on_pe=True,
           skip=frozenset()):
    import concourse.bass as bass
    import concourse.bacc as bacc
    import concourse.mybir as mybir
    import concourse.tile as tile

    F32 = mybir.dt.float32
    F32R = mybir.dt.float32r
    F16 = mybir.dt.float16
    AF = mybir.ActivationFunctionType
    OP = mybir.AluOpType
    AX = mybir.AxisListType

    nc = bacc.Bacc("TRN2", target_bir_lowering=False, debug=False,
                   enable_asserts=False, num_devices=num_devices)

    # host pre-arranged layout: [pair, partition(p2*64+c), t, x]
    dec_d = nc.dram_tensor("dec", [NPAIR, 128, T, BL], F32R,
                           kind="ExternalInput")
    enc_d = nc.dram_tensor("enc", [NPAIR, 128, T, BL], F32R,
                           kind="ExternalInput")
    # all parameters packed into one tensor: one DMA at startup
    # cols 0:128 wd | 128:256 we | 256:384 idp | 384:512 idm
    # | 512 bgh | 513 nth | 514 gam | 515 bet
    par_d = nc.dram_tensor("par", [128, 516], F32R, kind="ExternalInput")
    out_d = nc.dram_tensor("out", [NPAIR, 128, T, BL], F16,
                           kind="ExternalOutput")

    with tile.TileContext(nc) as tc:
        with tc.tile_pool(name="const", bufs=1) as cp, \
             tc.tile_pool(name="io", bufs=3) as io, \
             tc.tile_pool(name="wk", bufs=3) as wk, \
             tc.tile_pool(name="wkg", bufs=2) as wkg, \
             tc.tile_pool(name="wkf", bufs=3) as wkf, \
             tc.tile_pool(name="vv", bufs=4) as vv, \
             tc.tile_pool(name="sm", bufs=6) as sm, \
             tc.tile_pool(name="ot", bufs=6) as op_, \
             tc.tile_pool(name="ps", bufs=1, space="PSUM") as ps, \
             tc.tile_pool(name="psd", bufs=1, space="PSUM") as psd, \
             tc.tile_pool(name="dram", bufs=2, space="DRAM") as dp:

            par_t = cp.tile([128, 516], F32R)
            nc.sync.dma_start(par_t[:], par_d[:, :])
            wd_t = par_t[:, 0:128]
            we_t = par_t[:, 128:256]
            idp_t = par_t[:, 256:384]
            idm_t = par_t[:, 384:512]
            bgh_t = par_t[:, 512:513].bitcast(F32)
            nth_t = par_t[:, 513:514].bitcast(F32)
            gam_t = par_t[:, 514:515].bitcast(F32)
            bet_t = par_t[:, 515:516].bitcast(F32)

            store = cp.tile([128, NTILE, BL], F16)    # sign record
            scol = cp.tile([128, NTILE], F32)         # per-tile sign sums

            for _rep in range(reps):
                # ---------------- pass 1 (software-pipelined) ----------------
                # Stage lags per emission iteration k:
                #   dma(k); pre/h/D(k-1); gD/F(k-2); lif(k-3); signs(k-4/k-3)
                # Every instruction's inputs were produced in an earlier
                # iteration, so no engine stream ever head-of-line blocks.
                dec4s, enc4s = {}, {}
                h4s, gD4s, F4s = {}, {}, {}
                vps = {}      # pair -> list of v tiles (per t)

                def emit_dma(p):
                    # halves so the first matmuls unblock after 1/4 of the
                    # pair's bytes (shortens pipeline fill)
                    dec4 = io.tile([128, T, BL], F32R)
                    enc4 = io.tile([128, T, BL], F32R)
                    nc.sync.dma_start(dec4[:, 0:2], dec_d[p, :, 0:2])
                    nc.gpsimd.dma_start(enc4[:, 0:2], enc_d[p, :, 0:2])
                    nc.sync.dma_start(dec4[:, 2:4], dec_d[p, :, 2:4])
                    nc.gpsimd.dma_start(enc4[:, 2:4], enc_d[p, :, 2:4])
                    dec4s[p], enc4s[p] = dec4, enc4

                def emit_signs(p):
                    # sign for (p, 1..3) plus (p+1, 0): all deps one iter old
                    for t in range(1, T):
                        if 0 <= p < NPAIR:
                            idx = p * T + t
                            nc.scalar.activation(
                                store[:, idx], vps[p][t], AF.Sign,
                                bias=nth_t, scale=1.0,
                                accum_out=scol[:, idx:idx + 1])
                    q = p + 1
                    if 0 <= q < NPAIR:
                        idx = q * T
                        nc.scalar.activation(
                            store[:, idx], F4s[q][:, 0], AF.Sign,
                            bias=nth_t, scale=1.0,
                            accum_out=scol[:, idx:idx + 1])

                def emit_pre_h(p):
                    dec4, enc4 = dec4s[p], enc4s[p]
                    P4 = ps.tile([128, T, 512], F32)
                    for t in range(T):
                        nc.tensor.matmul(out=P4[:, t, 0:BL], lhsT=wd_t,
                                         rhs=dec4[:, t], start=True,
                                         stop=False)
                        nc.tensor.matmul(out=P4[:, t, 0:BL], lhsT=we_t,
                                         rhs=enc4[:, t], start=False,
                                         stop=True)
                    h4 = wk.tile([128, T, BL], F32)
                    nc.scalar.activation(h4[:], P4[:, :, 0:BL], AF.Tanh,
                                         bias=bgh_t, scale=0.5)
                    h4s[p] = h4

                def emit_D(p):
                    dec4, enc4 = dec4s[p], enc4s[p]
                    D4ap = psd.tile([128, T, 512], F32)
                    for t in range(T):
                        nc.tensor.matmul(out=D4ap[:, t, 0:BL],
                                         lhsT=idp_t, rhs=dec4[:, t],
                                         start=True, stop=False)
                        nc.tensor.matmul(out=D4ap[:, t, 0:BL],
                                         lhsT=idm_t, rhs=enc4[:, t],
                                         start=False, stop=True)
                    return D4ap

                def emit_gD_F(p, D4ap):
                    # gD = (1+h)*0.5*(dec-enc) = sigma(pre)*(dec-enc)
                    gD4 = wkg.tile([128, T, BL], F32)
                    nc.vector.scalar_tensor_tensor(out=gD4[:], in0=h4s[p][:],
                                                   scalar=1.0,
                                                   in1=D4ap[:, :, 0:BL],
                                                   op0=OP.add, op1=OP.mult)
                    F4 = wkf.tile([128, T, BL], F32)
                    nc.vector.scalar_tensor_tensor(out=F4[:], in0=gD4[:],
                                                   scalar=1.0,
                                                   in1=enc4s[p][:].bitcast(F32),
                                                   op0=OP.mult, op1=OP.add)
                    F4s[p] = F4
                    del h4s[p]
                    del dec4s[p], enc4s[p]

                def emit_lif(p):
                    F4 = F4s[p]
                    vp = F4[:, 0]
                    vlist = [vp]
                    for t in range(T - 1):
                        vrn = vv.tile([128, BL], F32)
                        nc.vector.scalar_tensor_tensor(
                            out=vrn[:], in0=vp, scalar=TH, in1=vp,
                            op0=OP.is_lt, op1=OP.mult)
                        vpt = sm.tile([128, BL], F32)
                        nc.vector.scalar_tensor_tensor(
                            out=vpt[:], in0=vrn[:], scalar=0.5,
                            in1=F4[:, t + 1], op0=OP.mult, op1=OP.add)
                        vp = vpt[:]
                        vlist.append(vp)
                    vps[p] = vlist

                D4prev = {}
                for k in range(NPAIR + 4):
                    if k >= 3:
                        emit_signs(k - 4)   # signs for pair k-4 (t>=1)
                                            # and pair k-3 (t=0)
                    if k < NPAIR:
                        emit_dma(k)
                    if 0 <= k - 1 < NPAIR:
                        emit_pre_h(k - 1)
                        D4prev[k - 1] = emit_D(k - 1)
                    if 0 <= k - 2 < NPAIR:
                        emit_gD_F(k - 2, D4prev.pop(k - 2))
                    if 0 <= k - 3 < NPAIR:
                        emit_lif(k - 3)

                # ---------------- stats ----------------
                # per-channel sign sum: contract the two 64-partition halves
                # of scol with a stacked identity on PE, then reduce tiles
                red = cp.tile([128, 1], F32)
                nc.vector.tensor_reduce(out=red[:], in_=scol[:, 0:NTILE],
                                        axis=AX.X, op=OP.add)
                redB = cp.tile([64, 1], F32)
                nc.sync.dma_start(redB[:], red[64:128, :])
                s64 = cp.tile([64, 1], F32)
                nc.vector.tensor_tensor(s64[:], red[0:64, :], redB[:], OP.add)
                # local spike count = 0.5*sum_sign + N_CORE/2
                loc = cp.tile([64, 1], F32)
                nc.vector.tensor_scalar(out=loc[:], in0=s64[:], scalar1=0.5,
                                        scalar2=float(N_CORE) / 2.0,
                                        op0=OP.mult, op1=OP.add)
                S128 = cp.tile([128, 1], F32)
                if use_collective:
                    cin = dp.tile([64, 1], F32)
                    cout = dp.tile([64, 1], F32)
                    nc.sync.dma_start(cin[:], loc[:])
                    nc.gpsimd.collective_compute(
                        "AllReduce", OP.add,
                        replica_groups=[list(range(num_devices))],
                        ins=[cin.opt()], outs=[cout.opt()])
                    nc.sync.dma_start(S128[0:64, :], cout[:])
                    nc.sync.dma_start(S128[64:128, :], cout[:])
                else:
                    Sl = cp.tile([64, 1], F32)
                    nc.vector.tensor_scalar(out=Sl[:], in0=loc[:],
                                            scalar1=float(NCORES),
                                            scalar2=None, op0=OP.mult)
                    nc.sync.dma_start(S128[0:64, :], Sl[:])
                    nc.sync.dma_start(S128[64:128, :], Sl[:])

                # all remaining stats math on 128 partitions directly
                mu = cp.tile([128, 1], F32)
                nc.vector.tensor_scalar(out=mu[:], in0=S128[:],
                                        scalar1=1.0 / float(N_TOTAL),
                                        scalar2=None, op0=OP.mult)
                # x = mu*(1-mu) + eps
                m1 = cp.tile([128, 1], F32)
                nc.vector.tensor_scalar(out=m1[:], in0=mu[:], scalar1=-1.0,
                                        scalar2=1.0, op0=OP.mult, op1=OP.add)
                x = cp.tile([128, 1], F32)
                nc.vector.tensor_tensor(x[:], m1[:], mu[:], OP.mult)
                nc.vector.tensor_scalar(out=x[:], in0=x[:], scalar1=EPS,
                                        scalar2=None, op0=OP.add)
                # r = 1/sqrt(x) + one Newton step r *= 1.5-0.5*x*r^2
                sq = cp.tile([128, 1], F32)
                nc.scalar.activation(sq[:], x[:], AF.Sqrt)
                r0 = cp.tile([128, 1], F32)
                nc.vector.reciprocal(r0[:], sq[:])
                e = cp.tile([128, 1], F32)
                nc.vector.tensor_tensor(e[:], r0[:], r0[:], OP.mult)
                nc.vector.tensor_tensor(e[:], e[:], x[:], OP.mult)
                nc.vector.tensor_scalar(out=e[:], in0=e[:], scalar1=-0.5,
                                        scalar2=1.5, op0=OP.mult, op1=OP.add)
                r = cp.tile([128, 1], F32)
                nc.vector.tensor_tensor(r[:], r0[:], e[:], OP.mult)
                # a = gamma*r ; scale = a/2 ; bias = a/2 + beta - mu*a
                a = cp.tile([128, 1], F32)
                nc.vector.tensor_tensor(a[:], gam_t, r[:], OP.mult)
                sc128 = cp.tile([128, 1], F32)
                nc.vector.tensor_scalar(out=sc128[:], in0=a[:], scalar1=0.5,
                                        scalar2=None, op0=OP.mult)
                tmp = cp.tile([128, 1], F32)
                nc.vector.tensor_tensor(tmp[:], mu[:], a[:], OP.mult)
                b0 = cp.tile([128, 1], F32)
                nc.vector.tensor_tensor(b0[:], bet_t, tmp[:], OP.subtract)
                bi128 = cp.tile([128, 1], F32)
                nc.vector.tensor_tensor(bi128[:], sc128[:], b0[:], OP.add)

                # ---------------- pass 2 ----------------
                for pair in range(NPAIR):
                    ot = op_.tile([128, T, BL], F16)
                    nc.vector.tensor_scalar(
                        out=ot[:], in0=store[:, pair * T:(pair + 1) * T, :],
                        scalar1=sc128[:], scalar2=bi128[:],
                        op0=OP.mult, op1=OP.add)
                    eng = nc.sync if pair % 2 == 0 else nc.gpsimd
                    eng.dma_start(out_d[pair], ot[:])

    nc.compile()
    return nc


def _prep_host(dec, enc, Wg, bg, gamma, beta):
    Wg = np.asarray(Wg, dtype=np.float32)
    wdT = np.ascontiguousarray(Wg[:, :64].T)   # [k, m] dec-part
    weT = np.ascontiguousarray(Wg[:, 64:].T)   # enc-part
    wd = np.zeros((128, 128), dtype=np.float32)
    we = np.zeros((128, 128), dtype=np.float32)
    wd[:64, :64] = wdT
    wd[64:, 64:] = wdT
    we[:64, :64] = weT
    we[64:, 64:] = weT
    bgh = np.tile(0.5 * np.asarray(bg, np.float32), 2)
    idp = np.eye(128, dtype=np.float32) * 0.5
    idm = np.eye(128, dtype=np.float32) * -0.5

    def relayout(x):
        # [T, C, NPIX] -> [pair, p2*64+c, t, x448]
        x = np.asarray(x, np.float32).reshape(T, C, NPAIR, 2, BL)
        return np.ascontiguousarray(x.transpose(2, 3, 1, 0, 4)
                                    .reshape(NPAIR, 128, T, BL))
    par = np.zeros((128, 516), dtype=np.float32)
    par[:, 0:128] = wd
    par[:, 128:256] = we
    par[:, 256:384] = idp
    par[:, 384:512] = idm
    par[:, 512] = bgh
    par[:, 513] = -TH
    par[:, 514] = np.tile(np.asarray(gamma, np.float32), 2)
    par[:, 515] = np.tile(np.asarray(beta, np.float32), 2)
    in_maps = []
    for b in range(NCORES):
        in_maps.append({
            "dec": relayout(np.asarray(dec[:, b]).reshape(T, C, NPIX)),
            "enc": relayout(np.asarray(enc[:, b]).reshape(T, C, NPIX)),
            "par": par,
        })
    return in_maps


def kernel(dec, enc, Wg, bg, gamma, beta, _trace=False, _trace_kwargs=None):
    from concourse.bass_utils import run_bass_kernel_spmd

    if "nc" not in _cache:
        _cache["nc"] = _build()
    nc = _cache["nc"]

    in_maps = _prep_host(dec, enc, Wg, bg, gamma, beta)
    kw = {}
    if _trace:
        kw["trace"] = True
        if _trace_kwargs:
            kw.update(_trace_kwargs)
    res = run_bass_kernel_spmd(nc, in_maps, core_ids=list(range(NCORES)), **kw)
    outs = []
    for b in range(NCORES):
        o = np.asarray(res.results[b]["out"]).astype(np.float32)
        # [pair, p2*64+c, t, x448] -> [T, C, NPIX]
        o = o.reshape(NPAIR, 2, C, T, BL).transpose(3, 2, 0, 1, 4)
        outs.append(o.reshape(T, C, NPIX))
    out = np.stack(outs, axis=1).reshape(T, B, C, H, W)
    if _trace:
        _cache["last_res"] = res
    return out


# revision 47
# speedup vs baseline: 1.6128x; 1.0071x over previous
"""Trainium2 Bass kernel for GatedSkipFusion (gate conv -> sigmoid blend ->
4-step LIF -> BatchNorm with training stats).

Self-contained: hardcodes shapes T=4, B=8, C=64, H=W=112; shards batch B
across 8 NeuronCores; BN stats via a 64-float AllReduce.

Math:
  gate = sigmoid(pre); fused = enc + gate*(dec-enc). With h = tanh(pre/2):
  gate = 0.5 + 0.5*h, so fused = enc + 0.5*(1+h)*D, D = dec-enc.
  LIF (tau=2, hard reset, v_th=0.15): v_t = 0.5*v_{t-1}*m_{t-1} + fused_t,
  m = (v < th). Spikes are binary so BN var = mu - mu^2; the BN output is a
  per-channel affine of the sign record sg = Sign(v - th) in {-1,0,1}:
  out = (a/2)*sg + (a/2 + beta - mu*a), a = gamma*rsqrt(var+eps).

Engine split (all four compute engines + DMA overlap; the per-core program
is memory-bound at ~360 GB/s):
  PE    : gate matmuls in fp32r (1 cyc/row)
  Act   : batched tanh from a 4-bank PSUM tile; per-step Sign with
          accumulation for the BN statistics
  DVE   : gD=(1+h)*D, F=0.5*gD+enc, reset-mask mult, and the final
          affine as a 4x-mode fp16 tensor_scalar
  Pool  : D=dec-enc and the v-update scalar_tensor_tensor
  fp16 output (halves the output DMA; ~6e-4 systematic error).
"""

import numpy as np

T, B, C, H, W = 4, 8, 64, 112, 112
NPIX = H * W          # 12544
BL = 448              # pixel block (free dim)
NPAIR = NPIX // (2 * BL)   # 14 pairs of blocks
NTILE = NPAIR * T     # 56 (pair,t) tiles
TH = 0.15
EPS = 1e-5
NCORES = 8
N_TOTAL = T * B * NPIX     # 401408 per-channel element count
N_CORE = T * NPIX          # 50176 per-core per-channel count

_cache = {}


def _build(reps=1, use_collective=True, num_devices=NCORES, d_on_pe=True,
           skip=frozenset()):
    import concourse.bass as bass
    import concourse.bacc as bacc
    import concourse.mybir as mybir
    import concourse.tile as tile

    F32 = mybir.dt.float32
    F32R = mybir.dt.float32r
    F16 = mybir.dt.float16
    AF = mybir.ActivationFunctionType
    OP = mybir.AluOpType
    AX = mybir.AxisListType

    nc = bacc.Bacc("TRN2", target_bir_lowering=False, debug=False,
                   enable_asserts=False, num_devices=num_devices)

    # host pre-arranged layout: [pair, partition(p2*64+c), t, x]
    dec_d = nc.dram_tensor("dec", [NPAIR, 128, T, BL], F32R,
                           kind="ExternalInput")
    enc_d = nc.dram_tensor("enc", [NPAIR, 128, T, BL], F32R,
                           kind="ExternalInput")
    # all parameters packed into one tensor: one DMA at startup
    # cols 0:128 wd | 128:256 we | 256:384 idp | 384:512 idm
    # | 512 bgh | 513 nth | 514 gam | 515 bet
    par_d = nc.dram_tensor("par", [128, 516], F32R, kind="ExternalInput")
    out_d = nc.dram_tensor("out", [NPAIR, 128, T, BL], F16,
                           kind="ExternalOutput")

    with tile.TileContext(nc) as tc:
        with tc.tile_pool(name="const", bufs=1) as cp, \
             tc.tile_pool(name="io", bufs=3) as io, \
             tc.tile_pool(name="wk", bufs=3) as wk, \
             tc.tile_pool(name="wkg", bufs=2) as wkg, \
             tc.tile_pool(name="wkf", bufs=3) as wkf, \
             tc.tile_pool(name="vv", bufs=4) as vv, \
             tc.tile_pool(name="sm", bufs=6) as sm, \
             tc.tile_pool(name="ot", bufs=6) as op_, \
             tc.tile_pool(name="ps", bufs=1, space="PSUM") as ps, \
             tc.tile_pool(name="psd", bufs=1, space="PSUM") as psd, \
             tc.tile_pool(name="dram", bufs=2, space="DRAM") as dp:

            par_t = cp.tile([128, 516], F32R)
            nc.sync.dma_start(par_t[:], par_d[:, :])
            wd_t = par_t[:, 0:128]
            we_t = par_t[:, 128:256]
            idp_t = par_t[:, 256:384]
            idm_t = par_t[:, 384:512]
            bgh_t = par_t[:, 512:513].bitcast(F32)
            nth_t = par_t[:, 513:514].bitcast(F32)
            gam_t = par_t[:, 514:515].bitcast(F32)
            bet_t = par_t[:, 515:516].bitcast(F32)

            store = cp.tile([128, NTILE, BL], F16)    # sign record
            scol = cp.tile([128, NTILE], F32)         # per-tile sign sums

            for _rep in range(reps):
                # ---------------- pass 1 (software-pipelined) ----------------
                # Stage lags per emission iteration k:
                #   dma(k); pre/h/D(k-1); gD/F(k-2); lif(k-3); signs(k-4/k-3)
                # Every instruction's inputs were produced in an earlier
                # iteration, so no engine stream ever head-of-line blocks.
                dec4s, enc4s = {}, {}
                h4s, gD4s, F4s = {}, {}, {}
                vps = {}      # pair -> list of v tiles (per t)

                def emit_dma(p):
                    # halves so the first matmuls unblock after 1/4 of the
                    # pair's bytes (shortens pipeline fill)
                    dec4 = io.tile([128, T, BL], F32R)
                    enc4 = io.tile([128, T, BL], F32R)
                    # first pairs on the hardware DGE only (shorter latency
                    # during pipeline fill); steady state splits across
                    # SP and Pool queues
                    enc_q = nc.sync if p < 2 else nc.gpsimd
                    nc.sync.dma_start(dec4[:, 0:2], dec_d[p, :, 0:2])
                    enc_q.dma_start(enc4[:, 0:2], enc_d[p, :, 0:2])
                    nc.sync.dma_start(dec4[:, 2:4], dec_d[p, :, 2:4])
                    enc_q.dma_start(enc4[:, 2:4], enc_d[p, :, 2:4])
                    dec4s[p], enc4s[p] = dec4, enc4

                def emit_signs(p):
                    # sign for (p, 1..3) plus (p+1, 0): all deps one iter old
                    for t in range(1, T):
                        if 0 <= p < NPAIR:
                            idx = p * T + t
                            nc.scalar.activation(
                                store[:, idx], vps[p][t], AF.Sign,
                                bias=nth_t, scale=1.0,
                                accum_out=scol[:, idx:idx + 1])
                    q = p + 1
                    if 0 <= q < NPAIR:
                        idx = q * T
                        nc.scalar.activation(
                            store[:, idx], F4s[q][:, 0], AF.Sign,
                            bias=nth_t, scale=1.0,
                            accum_out=scol[:, idx:idx + 1])

                def emit_pre_h(p):
                    dec4, enc4 = dec4s[p], enc4s[p]
                    P4 = ps.tile([128, T, 512], F32)
                    for t in range(T):
                        nc.tensor.matmul(out=P4[:, t, 0:BL], lhsT=wd_t,
                                         rhs=dec4[:, t], start=True,
                                         stop=False)
                        nc.tensor.matmul(out=P4[:, t, 0:BL], lhsT=we_t,
                                         rhs=enc4[:, t], start=False,
                                         stop=True)
                    h4 = wk.tile([128, T, BL], F32)
                    nc.scalar.activation(h4[:], P4[:, :, 0:BL], AF.Tanh,
                                         bias=bgh_t, scale=0.5)
                    h4s[p] = h4

                def emit_D(p):
                    dec4, enc4 = dec4s[p], enc4s[p]
                    D4ap = psd.tile([128, T, 512], F32)
                    for t in range(T):
                        nc.tensor.matmul(out=D4ap[:, t, 0:BL],
                                         lhsT=idp_t, rhs=dec4[:, t],
                                         start=True, stop=False)
                        nc.tensor.matmul(out=D4ap[:, t, 0:BL],
                                         lhsT=idm_t, rhs=enc4[:, t],
                                         start=False, stop=True)
                    return D4ap

                def emit_gD_F(p, D4ap):
                    # gD = (1+h)*0.5*(dec-enc) = sigma(pre)*(dec-enc)
                    gD4 = wkg.tile([128, T, BL], F32)
                    nc.vector.scalar_tensor_tensor(out=gD4[:], in0=h4s[p][:],
                                                   scalar=1.0,
                                                   in1=D4ap[:, :, 0:BL],
                                                   op0=OP.add, op1=OP.mult)
                    F4 = wkf.tile([128, T, BL], F32)
                    nc.vector.scalar_tensor_tensor(out=F4[:], in0=gD4[:],
                                                   scalar=1.0,
                                                   in1=enc4s[p][:].bitcast(F32),
                                                   op0=OP.mult, op1=OP.add)
                    F4s[p] = F4
                    del h4s[p]
                    del dec4s[p], enc4s[p]

                def emit_lif(p):
                    F4 = F4s[p]
                    vp = F4[:, 0]
                    vlist = [vp]
                    for t in range(T - 1):
                        vrn = vv.tile([128, BL], F32)
                        nc.vector.scalar_tensor_tensor(
                            out=vrn[:], in0=vp, scalar=TH, in1=vp,
                            op0=OP.is_lt, op1=OP.mult)
                        vpt = sm.tile([128, BL], F32)
                        nc.vector.scalar_tensor_tensor(
                            out=vpt[:], in0=vrn[:], scalar=0.5,
                            in1=F4[:, t + 1], op0=OP.mult, op1=OP.add)
                        vp = vpt[:]
                        vlist.append(vp)
                    vps[p] = vlist

                D4prev = {}
                for k in range(NPAIR + 4):
                    if k >= 3:
                        emit_signs(k - 4)   # signs for pair k-4 (t>=1)
                                            # and pair k-3 (t=0)
                    if k < NPAIR:
                        emit_dma(k)
                    if 0 <= k - 1 < NPAIR:
                        emit_pre_h(k - 1)
                        D4prev[k - 1] = emit_D(k - 1)
                    if 0 <= k - 2 < NPAIR:
                        emit_gD_F(k - 2, D4prev.pop(k - 2))
                    if 0 <= k - 3 < NPAIR:
                        emit_lif(k - 3)

                # ---------------- stats ----------------
                # per-channel sign sum: contract the two 64-partition halves
                # of scol with a stacked identity on PE, then reduce tiles
                red = cp.tile([128, 1], F32)
                nc.vector.tensor_reduce(out=red[:], in_=scol[:, 0:NTILE],
                                        axis=AX.X, op=OP.add)
                # swap halves with two parallel SBUF DMAs, then add:
                # s128sum[p] = per-channel total sign sum, duplicated
                red2 = cp.tile([128, 1], F32)
                nc.sync.dma_start(red2[0:64, :], red[64:128, :])
                nc.gpsimd.dma_start(red2[64:128, :], red[0:64, :])
                s128 = cp.tile([128, 1], F32)
                nc.vector.tensor_tensor(s128[:], red[:], red2[:], OP.add)
                mu = cp.tile([128, 1], F32)
                if use_collective:
                    # local spike count = 0.5*sum_sign + N_CORE/2
                    loc = cp.tile([64, 1], F32)
                    nc.vector.tensor_scalar(out=loc[:], in0=s128[0:64, :],
                                            scalar1=0.5,
                                            scalar2=float(N_CORE) / 2.0,
                                            op0=OP.mult, op1=OP.add)
                    cin = dp.tile([64, 1], F32)
                    cout = dp.tile([64, 1], F32)
                    nc.sync.dma_start(cin[:], loc[:])
                    nc.gpsimd.collective_compute(
                        "AllReduce", OP.add,
                        replica_groups=[list(range(num_devices))],
                        ins=[cin.opt()], outs=[cout.opt()])
                    S128 = cp.tile([128, 1], F32)
                    nc.sync.dma_start(S128[0:64, :], cout[:])
                    nc.gpsimd.dma_start(S128[64:128, :], cout[:])
                    nc.vector.tensor_scalar(out=mu[:], in0=S128[:],
                                            scalar1=1.0 / float(N_TOTAL),
                                            scalar2=None, op0=OP.mult)
                else:
                    # mu = ((0.5*sum + N_CORE/2) * NCORES) / N_TOTAL
                    nc.vector.tensor_scalar(
                        out=mu[:], in0=s128[:],
                        scalar1=0.5 * NCORES / float(N_TOTAL),
                        scalar2=N_CORE * 0.5 * NCORES / float(N_TOTAL),
                        op0=OP.mult, op1=OP.add)
                # x = mu*(1-mu) + eps
                m1 = cp.tile([128, 1], F32)
                nc.vector.tensor_scalar(out=m1[:], in0=mu[:], scalar1=-1.0,
                                        scalar2=1.0, op0=OP.mult, op1=OP.add)
                x = cp.tile([128, 1], F32)
                nc.vector.tensor_tensor(x[:], m1[:], mu[:], OP.mult)
                nc.vector.tensor_scalar(out=x[:], in0=x[:], scalar1=EPS,
                                        scalar2=None, op0=OP.add)
                # r = 1/sqrt(x) + one Newton step r *= 1.5-0.5*x*r^2
                sq = cp.tile([128, 1], F32)
                nc.scalar.activation(sq[:], x[:], AF.Sqrt)
                r0 = cp.tile([128, 1], F32)
                nc.vector.reciprocal(r0[:], sq[:])
                e = cp.tile([128, 1], F32)
                nc.vector.tensor_tensor(e[:], r0[:], r0[:], OP.mult)
                nc.vector.tensor_tensor(e[:], e[:], x[:], OP.mult)
                nc.vector.tensor_scalar(out=e[:], in0=e[:], scalar1=-0.5,
                                        scalar2=1.5, op0=OP.mult, op1=OP.add)
                r = cp.tile([128, 1], F32)
                nc.vector.tensor_tensor(r[:], r0[:], e[:], OP.mult)
                # a = gamma*r ; scale = a/2 ; bias = a/2 + beta - mu*a
                a = cp.tile([128, 1], F32)
                nc.vector.tensor_tensor(a[:], gam_t, r[:], OP.mult)
                sc128 = cp.tile([128, 1], F32)
                nc.vector.tensor_scalar(out=sc128[:], in0=a[:], scalar1=0.5,
                                        scalar2=None, op0=OP.mult)
                tmp = cp.tile([128, 1], F32)
                nc.vector.tensor_tensor(tmp[:], mu[:], a[:], OP.mult)
                b0 = cp.tile([128, 1], F32)
                nc.vector.tensor_tensor(b0[:], bet_t, tmp[:], OP.subtract)
                bi128 = cp.tile([128, 1], F32)
                nc.vector.tensor_tensor(bi128[:], sc128[:], b0[:], OP.add)

                # ---------------- pass 2 ----------------
                for pair in range(NPAIR):
                    ot = op_.tile([128, T, BL], F16)
                    nc.vector.tensor_scalar(
                        out=ot[:], in0=store[:, pair * T:(pair + 1) * T, :],
                        scalar1=sc128[:], scalar2=bi128[:],
                        op0=OP.mult, op1=OP.add)
                    eng = (nc.sync, nc.gpsimd, nc.scalar)[pair % 3]
                    eng.dma_start(out_d[pair], ot[:])

    nc.compile()
    return nc


def _prep_host(dec, enc, Wg, bg, gamma, beta):
    Wg = np.asarray(Wg, dtype=np.float32)
    wdT = np.ascontiguousarray(Wg[:, :64].T)   # [k, m] dec-part
    weT = np.ascontiguousarray(Wg[:, 64:].T)   # enc-part
    wd = np.zeros((128, 128), dtype=np.float32)
    we = np.zeros((128, 128), dtype=np.float32)
    wd[:64, :64] = wdT
    wd[64:, 64:] = wdT
    we[:64, :64] = weT
    we[64:, 64:] = weT
    bgh = np.tile(0.5 * np.asarray(bg, np.float32), 2)
    idp = np.eye(128, dtype=np.float32) * 0.5
    idm = np.eye(128, dtype=np.float32) * -0.5

    def relayout(x):
        # [T, C, NPIX] -> [pair, p2*64+c, t, x448]
        x = np.asarray(x, np.float32).reshape(T, C, NPAIR, 2, BL)
        return np.ascontiguousarray(x.transpose(2, 3, 1, 0, 4)
                                    .reshape(NPAIR, 128, T, BL))
    par = np.zeros((128, 516), dtype=np.float32)
    par[:, 0:128] = wd
    par[:, 128:256] = we
    par[:, 256:384] = idp
    par[:, 384:512] = idm
    par[:, 512] = bgh
    par[:, 513] = -TH
    par[:, 514] = np.tile(np.asarray(gamma, np.float32), 2)
    par[:, 515] = np.tile(np.asarray(beta, np.float32), 2)
    in_maps = []
    for b in range(NCORES):
        in_maps.append({
            "dec": relayout(np.asarray(dec[:, b]).reshape(T, C, NPIX)),
            "enc": relayout(np.asarray(enc[:, b]).reshape(T, C, NPIX)),
            "par": par,
        })
    return in_maps


def kernel(dec, enc, Wg, bg, gamma, beta, _trace=False, _trace_kwargs=None):
    from concourse.bass_utils import run_bass_kernel_spmd

    if "nc" not in _cache:
        _cache["nc"] = _build()
    nc = _cache["nc"]

    in_maps = _prep_host(dec, enc, Wg, bg, gamma, beta)
    kw = {}
    if _trace:
        kw["trace"] = True
        if _trace_kwargs:
            kw.update(_trace_kwargs)
    res = run_bass_kernel_spmd(nc, in_maps, core_ids=list(range(NCORES)), **kw)
    outs = []
    for b in range(NCORES):
        o = np.asarray(res.results[b]["out"]).astype(np.float32)
        # [pair, p2*64+c, t, x448] -> [T, C, NPIX]
        o = o.reshape(NPAIR, 2, C, T, BL).transpose(3, 2, 0, 1, 4)
        outs.append(o.reshape(T, C, NPIX))
    out = np.stack(outs, axis=1).reshape(T, B, C, H, W)
    if _trace:
        _cache["last_res"] = res
    return out


# revision 52
# speedup vs baseline: 1.6290x; 1.0101x over previous
"""Trainium2 Bass kernel for GatedSkipFusion (gate conv -> sigmoid blend ->
4-step LIF -> BatchNorm with training stats).

Self-contained: hardcodes shapes T=4, B=8, C=64, H=W=112; shards batch B
across 8 NeuronCores; BN stats via a 64-float AllReduce.

Math:
  gate = sigmoid(pre); fused = enc + gate*(dec-enc). With h = tanh(pre/2):
  gate = 0.5 + 0.5*h, so fused = enc + 0.5*(1+h)*D, D = dec-enc.
  LIF (tau=2, hard reset, v_th=0.15): v_t = 0.5*v_{t-1}*m_{t-1} + fused_t,
  m = (v < th). Spikes are binary so BN var = mu - mu^2; the BN output is a
  per-channel affine of the sign record sg = Sign(v - th) in {-1,0,1}:
  out = (a/2)*sg + (a/2 + beta - mu*a), a = gamma*rsqrt(var+eps).

Engine split (all four compute engines + DMA overlap; the per-core program
is memory-bound at ~360 GB/s):
  PE    : gate matmuls in fp32r (1 cyc/row)
  Act   : batched tanh from a 4-bank PSUM tile; per-step Sign with
          accumulation for the BN statistics
  DVE   : gD=(1+h)*D, F=0.5*gD+enc, reset-mask mult, and the final
          affine as a 4x-mode fp16 tensor_scalar
  Pool  : D=dec-enc and the v-update scalar_tensor_tensor
  fp16 output (halves the output DMA; ~6e-4 systematic error).
"""

import numpy as np

T, B, C, H, W = 4, 8, 64, 112, 112
NPIX = H * W          # 12544
BL = 448              # pixel block (free dim)
NPAIR = NPIX // (2 * BL)   # 14 pairs of blocks
NTILE = NPAIR * T     # 56 (pair,t) tiles
TH = 0.15
EPS = 1e-5
NCORES = 8
N_TOTAL = T * B * NPIX     # 401408 per-channel element count
N_CORE = T * NPIX          # 50176 per-core per-channel count

_cache = {}


def _build(reps=1, use_collective=True, num_devices=NCORES, d_on_pe=True,
           skip=frozenset()):
    import concourse.bass as bass
    import concourse.bacc as bacc
    import concourse.mybir as mybir
    import concourse.tile as tile

    F32 = mybir.dt.float32
    F32R = mybir.dt.float32r
    F16 = mybir.dt.float16
    AF = mybir.ActivationFunctionType
    OP = mybir.AluOpType
    AX = mybir.AxisListType

    nc = bacc.Bacc("TRN2", target_bir_lowering=False, debug=False,
                   enable_asserts=False, num_devices=num_devices)

    # host pre-arranged layout: [pair, partition(p2*64+c), t, x]
    dec_d = nc.dram_tensor("dec", [NPAIR, 128, T, BL], F32R,
                           kind="ExternalInput")
    enc_d = nc.dram_tensor("enc", [NPAIR, 128, T, BL], F32R,
                           kind="ExternalInput")
    # all parameters packed into one tensor: one DMA at startup
    # cols 0:128 wd | 128:256 we | 256:384 idp | 384:512 idm
    # | 512 bgh | 513 nth | 514 gam | 515 bet
    par_d = nc.dram_tensor("par", [128, 516], F32R, kind="ExternalInput")
    out_d = nc.dram_tensor("out", [NPAIR, 128, T, BL], F16,
                           kind="ExternalOutput")

    with tile.TileContext(nc) as tc:
        with tc.tile_pool(name="const", bufs=1) as cp, \
             tc.tile_pool(name="io", bufs=3) as io, \
             tc.tile_pool(name="wk", bufs=3) as wk, \
             tc.tile_pool(name="wkg", bufs=2) as wkg, \
             tc.tile_pool(name="wkf", bufs=3) as wkf, \
             tc.tile_pool(name="vv", bufs=3) as vv, \
             tc.tile_pool(name="sm", bufs=6) as sm, \
             tc.tile_pool(name="ot", bufs=5) as op_, \
             tc.tile_pool(name="ps", bufs=1, space="PSUM") as ps, \
             tc.tile_pool(name="psd", bufs=1, space="PSUM") as psd, \
             tc.tile_pool(name="dram", bufs=2, space="DRAM") as dp:

            par_t = cp.tile([128, 516], F32R)
            nc.sync.dma_start(par_t[:], par_d[:, :])
            wd_t = par_t[:, 0:128]
            we_t = par_t[:, 128:256]
            idp_t = par_t[:, 256:384]
            idm_t = par_t[:, 384:512]
            bgh_t = par_t[:, 512:513].bitcast(F32)
            nth_t = par_t[:, 513:514].bitcast(F32)
            gam_t = par_t[:, 514:515].bitcast(F32)
            bet_t = par_t[:, 515:516].bitcast(F32)

            store = cp.tile([128, NTILE, BL], F16)    # sign record
            scol = cp.tile([128, NTILE], F32)         # per-tile sign sums

            for _rep in range(reps):
                # ---------------- pass 1 (software-pipelined) ----------------
                # Stage lags per emission iteration k:
                #   dma(k); pre/h/D(k-1); gD/F(k-2); lif(k-3); signs(k-4/k-3)
                # Every instruction's inputs were produced in an earlier
                # iteration, so no engine stream ever head-of-line blocks.
                dec4s, enc4s = {}, {}
                h4s, gD4s, F4s = {}, {}, {}
                vps = {}      # pair -> list of v tiles (per t)

                def emit_dma(p):
                    # halves so the first matmuls unblock after 1/4 of the
                    # pair's bytes (shortens pipeline fill)
                    dec4 = io.tile([128, T, BL], F32R)
                    enc4 = io.tile([128, T, BL], F32R)
                    if p == 0:
                        # dec first: the gate matmuls only need dec
                        nc.sync.dma_start(dec4[:, 0:2], dec_d[p, :, 0:2])
                        nc.sync.dma_start(dec4[:, 2:4], dec_d[p, :, 2:4])
                        nc.sync.dma_start(enc4[:, 0:2], enc_d[p, :, 0:2])
                        nc.sync.dma_start(enc4[:, 2:4], enc_d[p, :, 2:4])
                    else:
                        nc.sync.dma_start(dec4[:, 0:2], dec_d[p, :, 0:2])
                        nc.sync.dma_start(enc4[:, 0:2], enc_d[p, :, 0:2])
                        nc.sync.dma_start(dec4[:, 2:4], dec_d[p, :, 2:4])
                        nc.sync.dma_start(enc4[:, 2:4], enc_d[p, :, 2:4])
                    dec4s[p], enc4s[p] = dec4, enc4

                def emit_signs(p):
                    # sign for (p, 1..3) plus (p+1, 0): all deps one iter old
                    for t in range(1, T):
                        if 0 <= p < NPAIR:
                            idx = p * T + t
                            nc.scalar.activation(
                                store[:, idx], vps[p][t], AF.Sign,
                                bias=nth_t, scale=1.0,
                                accum_out=scol[:, idx:idx + 1])
                    q = p + 1
                    if 0 <= q < NPAIR:
                        idx = q * T
                        nc.scalar.activation(
                            store[:, idx], F4s[q][:, 0], AF.Sign,
                            bias=nth_t, scale=1.0,
                            accum_out=scol[:, idx:idx + 1])

                def emit_pre_h(p):
                    dec4, enc4 = dec4s[p], enc4s[p]
                    P4 = ps.tile([128, T, 512], F32)
                    for t in range(T):
                        nc.tensor.matmul(out=P4[:, t, 0:BL], lhsT=wd_t,
                                         rhs=dec4[:, t], start=True,
                                         stop=False)
                        nc.tensor.matmul(out=P4[:, t, 0:BL], lhsT=we_t,
                                         rhs=enc4[:, t], start=False,
                                         stop=True)
                    h4 = wk.tile([128, T, BL], F32)
                    nc.scalar.activation(h4[:], P4[:, :, 0:BL], AF.Tanh,
                                         bias=bgh_t, scale=0.5)
                    h4s[p] = h4

                def emit_D(p):
                    dec4, enc4 = dec4s[p], enc4s[p]
                    if p < 2:
                        # fill phase: DVE is idle, and this keeps the PE +
                        # PSUM path off the critical startup chain
                        D4t = wk.tile([128, T, BL], F32)
                        nc.vector.tensor_tensor(D4t[:], dec4[:].bitcast(F32),
                                                enc4[:].bitcast(F32),
                                                OP.subtract)
                        return ("sbuf", D4t)
                    D4ap = psd.tile([128, T, 512], F32)
                    for t in range(T):
                        nc.tensor.matmul(out=D4ap[:, t, 0:BL],
                                         lhsT=idp_t, rhs=dec4[:, t],
                                         start=True, stop=False)
                        nc.tensor.matmul(out=D4ap[:, t, 0:BL],
                                         lhsT=idm_t, rhs=enc4[:, t],
                                         start=False, stop=True)
                    return ("psum", D4ap)

                def emit_gD_F(p, D4pack):
                    # gD = (1+h)*0.5*(dec-enc) = sigma(pre)*(dec-enc)
                    kind, D4ap = D4pack
                    if kind == "sbuf":
                        # D unscaled: fold the 0.5 into F's scalar instead
                        D4v, fscale = D4ap[:], 0.5
                    else:
                        D4v, fscale = D4ap[:, :, 0:BL], 1.0
                    gD4 = wkg.tile([128, T, BL], F32)
                    nc.vector.scalar_tensor_tensor(out=gD4[:], in0=h4s[p][:],
                                                   scalar=1.0,
                                                   in1=D4v,
                                                   op0=OP.add, op1=OP.mult)
                    F4 = wkf.tile([128, T, BL], F32)
                    nc.vector.scalar_tensor_tensor(out=F4[:], in0=gD4[:],
                                                   scalar=fscale,
                                                   in1=enc4s[p][:].bitcast(F32),
                                                   op0=OP.mult, op1=OP.add)
                    F4s[p] = F4
                    del h4s[p]
                    del dec4s[p], enc4s[p]

                def emit_lif(p):
                    F4 = F4s[p]
                    vp = F4[:, 0]
                    vlist = [vp]
                    for t in range(T - 1):
                        vrn = vv.tile([128, BL], F32)
                        nc.vector.scalar_tensor_tensor(
                            out=vrn[:], in0=vp, scalar=TH, in1=vp,
                            op0=OP.is_lt, op1=OP.mult)
                        vpt = sm.tile([128, BL], F32)
                        nc.vector.scalar_tensor_tensor(
                            out=vpt[:], in0=vrn[:], scalar=0.5,
                            in1=F4[:, t + 1], op0=OP.mult, op1=OP.add)
                        vp = vpt[:]
                        vlist.append(vp)
                    vps[p] = vlist

                D4prev = {}
                for k in range(NPAIR + 4):
                    if k >= 3:
                        emit_signs(k - 4)   # signs for pair k-4 (t>=1)
                                            # and pair k-3 (t=0)
                    if k < NPAIR:
                        emit_dma(k)
                    if 0 <= k - 1 < NPAIR:
                        emit_pre_h(k - 1)
                        D4prev[k - 1] = emit_D(k - 1)
                    if 0 <= k - 2 < NPAIR:
                        emit_gD_F(k - 2, D4prev.pop(k - 2))
                    if 0 <= k - 3 < NPAIR:
                        emit_lif(k - 3)

                # ---------------- stats ----------------
                # per-channel sign sum: contract the two 64-partition halves
                # of scol with a stacked identity on PE, then reduce tiles
                red = cp.tile([128, 1], F32)
                nc.vector.tensor_reduce(out=red[:], in_=scol[:, 0:NTILE],
                                        axis=AX.X, op=OP.add)
                # swap halves with two parallel SBUF DMAs, then add:
                # s128sum[p] = per-channel total sign sum, duplicated
                red2 = cp.tile([128, 1], F32)
                nc.sync.dma_start(red2[0:64, :], red[64:128, :])
                nc.gpsimd.dma_start(red2[64:128, :], red[0:64, :])
                s128 = cp.tile([128, 1], F32)
                nc.vector.tensor_tensor(s128[:], red[:], red2[:], OP.add)
                mu = cp.tile([128, 1], F32)
                if use_collective:
                    # local spike count = 0.5*sum_sign + N_CORE/2
                    loc = cp.tile([64, 1], F32)
                    nc.vector.tensor_scalar(out=loc[:], in0=s128[0:64, :],
                                            scalar1=0.5,
                                            scalar2=float(N_CORE) / 2.0,
                                            op0=OP.mult, op1=OP.add)
                    cin = dp.tile([64, 1], F32)
                    cout = dp.tile([64, 1], F32)
                    nc.sync.dma_start(cin[:], loc[:])
                    nc.gpsimd.collective_compute(
                        "AllReduce", OP.add,
                        replica_groups=[list(range(num_devices))],
                        ins=[cin.opt()], outs=[cout.opt()])
                    S128 = cp.tile([128, 1], F32)
                    nc.sync.dma_start(S128[0:64, :], cout[:])
                    nc.gpsimd.dma_start(S128[64:128, :], cout[:])
                    nc.vector.tensor_scalar(out=mu[:], in0=S128[:],
                                            scalar1=1.0 / float(N_TOTAL),
                                            scalar2=None, op0=OP.mult)
                else:
                    # mu = ((0.5*sum + N_CORE/2) * NCORES) / N_TOTAL
                    nc.vector.tensor_scalar(
                        out=mu[:], in0=s128[:],
                        scalar1=0.5 * NCORES / float(N_TOTAL),
                        scalar2=N_CORE * 0.5 * NCORES / float(N_TOTAL),
                        op0=OP.mult, op1=OP.add)
                # x = mu*(1-mu) + eps
                m1 = cp.tile([128, 1], F32)
                nc.vector.tensor_scalar(out=m1[:], in0=mu[:], scalar1=-1.0,
                                        scalar2=1.0, op0=OP.mult, op1=OP.add)
                x = cp.tile([128, 1], F32)
                nc.vector.tensor_tensor(x[:], m1[:], mu[:], OP.mult)
                nc.vector.tensor_scalar(out=x[:], in0=x[:], scalar1=EPS,
                                        scalar2=None, op0=OP.add)
                # r = 1/sqrt(x) + one Newton step r *= 1.5-0.5*x*r^2
                sq = cp.tile([128, 1], F32)
                nc.scalar.activation(sq[:], x[:], AF.Sqrt)
                r0 = cp.tile([128, 1], F32)
                nc.vector.reciprocal(r0[:], sq[:])
                e = cp.tile([128, 1], F32)
                nc.vector.tensor_tensor(e[:], r0[:], r0[:], OP.mult)
                nc.vector.tensor_tensor(e[:], e[:], x[:], OP.mult)
                nc.vector.tensor_scalar(out=e[:], in0=e[:], scalar1=-0.5,
                                        scalar2=1.5, op0=OP.mult, op1=OP.add)
                r = cp.tile([128, 1], F32)
                nc.vector.tensor_tensor(r[:], r0[:], e[:], OP.mult)
                # a = gamma*r ; scale = a/2 ; bias = a/2 + beta - mu*a
                a = cp.tile([128, 1], F32)
                nc.vector.tensor_tensor(a[:], gam_t, r[:], OP.mult)
                sc128 = cp.tile([128, 1], F32)
                nc.vector.tensor_scalar(out=sc128[:], in0=a[:], scalar1=0.5,
                                        scalar2=None, op0=OP.mult)
                tmp = cp.tile([128, 1], F32)
                nc.vector.tensor_tensor(tmp[:], mu[:], a[:], OP.mult)
                b0 = cp.tile([128, 1], F32)
                nc.vector.tensor_tensor(b0[:], bet_t, tmp[:], OP.subtract)
                bi128 = cp.tile([128, 1], F32)
                nc.vector.tensor_tensor(bi128[:], sc128[:], b0[:], OP.add)

                # ---------------- pass 2 ----------------
                for pair in range(NPAIR):
                    ot = op_.tile([128, T, BL], F16)
                    nc.vector.tensor_scalar(
                        out=ot[:], in0=store[:, pair * T:(pair + 1) * T, :],
                        scalar1=sc128[:], scalar2=bi128[:],
                        op0=OP.mult, op1=OP.add)
                    eng = (nc.sync, nc.gpsimd, nc.scalar)[pair % 3]
                    eng.dma_start(out_d[pair], ot[:])

    nc.compile()
    return nc


def _prep_host(dec, enc, Wg, bg, gamma, beta):
    Wg = np.asarray(Wg, dtype=np.float32)
    wdT = np.ascontiguousarray(Wg[:, :64].T)   # [k, m] dec-part
    weT = np.ascontiguousarray(Wg[:, 64:].T)   # enc-part
    wd = np.zeros((128, 128), dtype=np.float32)
    we = np.zeros((128, 128), dtype=np.float32)
    wd[:64, :64] = wdT
    wd[64:, 64:] = wdT
    we[:64, :64] = weT
    we[64:, 64:] = weT
    bgh = np.tile(0.5 * np.asarray(bg, np.float32), 2)
    idp = np.eye(128, dtype=np.float32) * 0.5
    idm = np.eye(128, dtype=np.float32) * -0.5

    def relayout(x):
        # [T, C, NPIX] -> [pair, p2*64+c, t, x448]
        x = np.asarray(x, np.float32).reshape(T, C, NPAIR, 2, BL)
        return np.ascontiguousarray(x.transpose(2, 3, 1, 0, 4)
                                    .reshape(NPAIR, 128, T, BL))
    par = np.zeros((128, 516), dtype=np.float32)
    par[:, 0:128] = wd
    par[:, 128:256] = we
    par[:, 256:384] = idp
    par[:, 384:512] = idm
    par[:, 512] = bgh
    par[:, 513] = -TH
    par[:, 514] = np.tile(np.asarray(gamma, np.float32), 2)
    par[:, 515] = np.tile(np.asarray(beta, np.float32), 2)
    in_maps = []
    for b in range(NCORES):
        in_maps.append({
            "dec": relayout(np.asarray(dec[:, b]).reshape(T, C, NPIX)),
            "enc": relayout(np.asarray(enc[:, b]).reshape(T, C, NPIX)),
            "par": par,
        })
    return in_maps


def kernel(dec, enc, Wg, bg, gamma, beta, _trace=False, _trace_kwargs=None):
    from concourse.bass_utils import run_bass_kernel_spmd

    if "nc" not in _cache:
        _cache["nc"] = _build()
    nc = _cache["nc"]

    in_maps = _prep_host(dec, enc, Wg, bg, gamma, beta)
    kw = {}
    if _trace:
        kw["trace"] = True
        if _trace_kwargs:
            kw.update(_trace_kwargs)
    res = run_bass_kernel_spmd(nc, in_maps, core_ids=list(range(NCORES)), **kw)
    outs = []
    for b in range(NCORES):
        o = np.asarray(res.results[b]["out"]).astype(np.float32)
        # [pair, p2*64+c, t, x448] -> [T, C, NPIX]
        o = o.reshape(NPAIR, 2, C, T, BL).transpose(3, 2, 0, 1, 4)
        outs.append(o.reshape(T, C, NPIX))
    out = np.stack(outs, axis=1).reshape(T, B, C, H, W)
    if _trace:
        _cache["last_res"] = res
    return out


# revision 55
# speedup vs baseline: 1.6369x; 1.0048x over previous
"""Trainium2 Bass kernel for GatedSkipFusion (gate conv -> sigmoid blend ->
4-step LIF -> BatchNorm with training stats).

Self-contained: hardcodes shapes T=4, B=8, C=64, H=W=112; shards batch B
across 8 NeuronCores; BN stats via a 64-float AllReduce.

Math:
  gate = sigmoid(pre); fused = enc + gate*(dec-enc). With h = tanh(pre/2):
  gate = 0.5 + 0.5*h, so fused = enc + 0.5*(1+h)*D, D = dec-enc.
  LIF (tau=2, hard reset, v_th=0.15): v_t = 0.5*v_{t-1}*m_{t-1} + fused_t,
  m = (v < th). Spikes are binary so BN var = mu - mu^2; the BN output is a
  per-channel affine of the sign record sg = Sign(v - th) in {-1,0,1}:
  out = (a/2)*sg + (a/2 + beta - mu*a), a = gamma*rsqrt(var+eps).

Engine split (all four compute engines + DMA overlap; the per-core program
is memory-bound at ~360 GB/s):
  PE    : gate matmuls in fp32r (1 cyc/row)
  Act   : batched tanh from a 4-bank PSUM tile; per-step Sign with
          accumulation for the BN statistics
  DVE   : gD=(1+h)*D, F=0.5*gD+enc, reset-mask mult, and the final
          affine as a 4x-mode fp16 tensor_scalar
  Pool  : D=dec-enc and the v-update scalar_tensor_tensor
  fp16 output (halves the output DMA; ~6e-4 systematic error).
"""

import numpy as np

T, B, C, H, W = 4, 8, 64, 112, 112
NPIX = H * W          # 12544
BL = 448              # pixel block (free dim)
NPAIR = NPIX // (2 * BL)   # 14 pairs of blocks
NTILE = NPAIR * T     # 56 (pair,t) tiles
TH = 0.15
EPS = 1e-5
NCORES = 8
N_TOTAL = T * B * NPIX     # 401408 per-channel element count
N_CORE = T * NPIX          # 50176 per-core per-channel count

_cache = {}


def _build(reps=1, use_collective=True, num_devices=NCORES, d_on_pe=True,
           skip=frozenset()):
    import concourse.bass as bass
    import concourse.bacc as bacc
    import concourse.mybir as mybir
    import concourse.tile as tile

    F32 = mybir.dt.float32
    F32R = mybir.dt.float32r
    F16 = mybir.dt.float16
    AF = mybir.ActivationFunctionType
    OP = mybir.AluOpType
    AX = mybir.AxisListType

    nc = bacc.Bacc("TRN2", target_bir_lowering=False, debug=False,
                   enable_asserts=False, num_devices=num_devices)

    # host pre-arranged layout: [pair, partition(p2*64+c), t, x]
    dec_d = nc.dram_tensor("dec", [NPAIR, 128, T, BL], F32R,
                           kind="ExternalInput")
    enc_d = nc.dram_tensor("enc", [NPAIR, 128, T, BL], F32R,
                           kind="ExternalInput")
    # all parameters packed into one tensor: one DMA at startup
    # cols 0:128 wd | 128:256 we | 256:384 idp | 384:512 idm
    # | 512 bgh | 513 nth | 514 gam | 515 bet
    par_d = nc.dram_tensor("par", [128, 516], F32R, kind="ExternalInput")
    out_d = nc.dram_tensor("out", [NPAIR, 128, T, BL], F16,
                           kind="ExternalOutput")

    with tile.TileContext(nc) as tc:
        with tc.tile_pool(name="const", bufs=1) as cp, \
             tc.tile_pool(name="io", bufs=3) as io, \
             tc.tile_pool(name="wk", bufs=3) as wk, \
             tc.tile_pool(name="wkg", bufs=2) as wkg, \
             tc.tile_pool(name="wkf", bufs=3) as wkf, \
             tc.tile_pool(name="vv", bufs=3) as vv, \
             tc.tile_pool(name="sm", bufs=6) as sm, \
             tc.tile_pool(name="ot", bufs=5) as op_, \
             tc.tile_pool(name="ps", bufs=1, space="PSUM") as ps, \
             tc.tile_pool(name="psd", bufs=1, space="PSUM") as psd, \
             tc.tile_pool(name="dram", bufs=2, space="DRAM") as dp:

            par_t = cp.tile([128, 516], F32R)
            nc.sync.dma_start(par_t[:], par_d[:, :])
            wd_t = par_t[:, 0:128]
            we_t = par_t[:, 128:256]
            idp_t = par_t[:, 256:384]
            idm_t = par_t[:, 384:512]
            bgh_t = par_t[:, 512:513].bitcast(F32)
            nth_t = par_t[:, 513:514].bitcast(F32)
            gam_t = par_t[:, 514:515].bitcast(F32)
            bet_t = par_t[:, 515:516].bitcast(F32)

            store = cp.tile([128, NTILE, BL], F16)    # sign record
            scol = cp.tile([128, NTILE], F32)         # per-tile sign sums



            for _rep in range(reps):
                # ---------------- pass 1 (software-pipelined) ----------------
                # Stage lags per emission iteration k:
                #   dma(k); pre/h/D(k-1); gD/F(k-2); lif(k-3); signs(k-4/k-3)
                # Every instruction's inputs were produced in an earlier
                # iteration, so no engine stream ever head-of-line blocks.
                dec4s, enc4s = {}, {}
                h4s, gD4s, F4s = {}, {}, {}
                vps = {}      # pair -> list of v tiles (per t)

                def emit_dma(p):
                    # halves so the first matmuls unblock after 1/4 of the
                    # pair's bytes (shortens pipeline fill)
                    dec4 = io.tile([128, T, BL], F32R)
                    enc4 = io.tile([128, T, BL], F32R)
                    if p == 0:
                        # dec first: the gate matmuls only need dec
                        nc.sync.dma_start(dec4[:, 0:2], dec_d[p, :, 0:2])
                        nc.sync.dma_start(dec4[:, 2:4], dec_d[p, :, 2:4])
                        nc.sync.dma_start(enc4[:, 0:2], enc_d[p, :, 0:2])
                        nc.sync.dma_start(enc4[:, 2:4], enc_d[p, :, 2:4])
                    else:
                        nc.sync.dma_start(dec4[:, 0:2], dec_d[p, :, 0:2])
                        nc.sync.dma_start(enc4[:, 0:2], enc_d[p, :, 0:2])
                        nc.sync.dma_start(dec4[:, 2:4], dec_d[p, :, 2:4])
                        nc.sync.dma_start(enc4[:, 2:4], enc_d[p, :, 2:4])
                    dec4s[p], enc4s[p] = dec4, enc4

                def emit_signs(p):
                    # sign for (p, 1..3) plus (p+1, 0): all deps one iter old
                    for t in range(1, T):
                        if 0 <= p < NPAIR:
                            idx = p * T + t
                            nc.scalar.activation(
                                store[:, idx], vps[p][t], AF.Sign,
                                bias=nth_t, scale=1.0,
                                accum_out=scol[:, idx:idx + 1])
                    q = p + 1
                    if 0 <= q < NPAIR:
                        idx = q * T
                        nc.scalar.activation(
                            store[:, idx], F4s[q][:, 0], AF.Sign,
                            bias=nth_t, scale=1.0,
                            accum_out=scol[:, idx:idx + 1])

                def emit_pre_h(p):
                    dec4, enc4 = dec4s[p], enc4s[p]
                    P4 = ps.tile([128, T, 512], F32)
                    if p == 0:
                        # warm the PE p-state on the param tile while the
                        # first input DMAs stream; the real matmuls below
                        # overwrite these banks (start=True resets PSUM)
                        for w in range(6):
                            nc.tensor.matmul(out=P4[:, w % T, 0:BL],
                                             lhsT=idp_t,
                                             rhs=par_t[:, 0:448],
                                             start=True, stop=True)
                    for t in range(T):
                        nc.tensor.matmul(out=P4[:, t, 0:BL], lhsT=wd_t,
                                         rhs=dec4[:, t], start=True,
                                         stop=False)
                        nc.tensor.matmul(out=P4[:, t, 0:BL], lhsT=we_t,
                                         rhs=enc4[:, t], start=False,
                                         stop=True)
                    h4 = wk.tile([128, T, BL], F32)
                    nc.scalar.activation(h4[:], P4[:, :, 0:BL], AF.Tanh,
                                         bias=bgh_t, scale=0.5)
                    h4s[p] = h4

                def emit_D(p):
                    dec4, enc4 = dec4s[p], enc4s[p]
                    if p < 2:
                        # fill phase: DVE is idle, and this keeps the PE +
                        # PSUM path off the critical startup chain
                        D4t = wk.tile([128, T, BL], F32)
                        nc.vector.tensor_tensor(D4t[:], dec4[:].bitcast(F32),
                                                enc4[:].bitcast(F32),
                                                OP.subtract)
                        return ("sbuf", D4t)
                    D4ap = psd.tile([128, T, 512], F32)
                    for t in range(T):
                        nc.tensor.matmul(out=D4ap[:, t, 0:BL],
                                         lhsT=idp_t, rhs=dec4[:, t],
                                         start=True, stop=False)
                        nc.tensor.matmul(out=D4ap[:, t, 0:BL],
                                         lhsT=idm_t, rhs=enc4[:, t],
                                         start=False, stop=True)
                    return ("psum", D4ap)

                def emit_gD_F(p, D4pack):
                    # gD = (1+h)*0.5*(dec-enc) = sigma(pre)*(dec-enc)
                    kind, D4ap = D4pack
                    if kind == "sbuf":
                        # D unscaled: fold the 0.5 into F's scalar instead
                        D4v, fscale = D4ap[:], 0.5
                    else:
                        D4v, fscale = D4ap[:, :, 0:BL], 1.0
                    gD4 = wkg.tile([128, T, BL], F32)
                    nc.vector.scalar_tensor_tensor(out=gD4[:], in0=h4s[p][:],
                                                   scalar=1.0,
                                                   in1=D4v,
                                                   op0=OP.add, op1=OP.mult)
                    F4 = wkf.tile([128, T, BL], F32)
                    nc.vector.scalar_tensor_tensor(out=F4[:], in0=gD4[:],
                                                   scalar=fscale,
                                                   in1=enc4s[p][:].bitcast(F32),
                                                   op0=OP.mult, op1=OP.add)
                    F4s[p] = F4
                    del h4s[p]
                    del dec4s[p], enc4s[p]

                def emit_lif(p):
                    F4 = F4s[p]
                    vp = F4[:, 0]
                    vlist = [vp]
                    for t in range(T - 1):
                        vrn = vv.tile([128, BL], F32)
                        nc.vector.scalar_tensor_tensor(
                            out=vrn[:], in0=vp, scalar=TH, in1=vp,
                            op0=OP.is_lt, op1=OP.mult)
                        vpt = sm.tile([128, BL], F32)
                        nc.vector.scalar_tensor_tensor(
                            out=vpt[:], in0=vrn[:], scalar=0.5,
                            in1=F4[:, t + 1], op0=OP.mult, op1=OP.add)
                        vp = vpt[:]
                        vlist.append(vp)
                    vps[p] = vlist

                D4prev = {}
                for k in range(NPAIR + 4):
                    if k >= 3:
                        emit_signs(k - 4)   # signs for pair k-4 (t>=1)
                                            # and pair k-3 (t=0)
                    if k < NPAIR:
                        emit_dma(k)
                    if 0 <= k - 1 < NPAIR:
                        emit_pre_h(k - 1)
                        D4prev[k - 1] = emit_D(k - 1)
                    if 0 <= k - 2 < NPAIR:
                        emit_gD_F(k - 2, D4prev.pop(k - 2))
                    if 0 <= k - 3 < NPAIR:
                        emit_lif(k - 3)

                # ---------------- stats ----------------
                # per-channel sign sum: contract the two 64-partition halves
                # of scol with a stacked identity on PE, then reduce tiles
                red = cp.tile([128, 1], F32)
                nc.vector.tensor_reduce(out=red[:], in_=scol[:, 0:NTILE],
                                        axis=AX.X, op=OP.add)
                # swap halves with two parallel SBUF DMAs, then add:
                # s128sum[p] = per-channel total sign sum, duplicated
                red2 = cp.tile([128, 1], F32)
                nc.sync.dma_start(red2[0:64, :], red[64:128, :])
                nc.gpsimd.dma_start(red2[64:128, :], red[0:64, :])
                s128 = cp.tile([128, 1], F32)
                nc.vector.tensor_tensor(s128[:], red[:], red2[:], OP.add)
                mu = cp.tile([128, 1], F32)
                if use_collective:
                    # local spike count = 0.5*sum_sign + N_CORE/2
                    loc = cp.tile([64, 1], F32)
                    nc.vector.tensor_scalar(out=loc[:], in0=s128[0:64, :],
                                            scalar1=0.5,
                                            scalar2=float(N_CORE) / 2.0,
                                            op0=OP.mult, op1=OP.add)
                    cin = dp.tile([64, 1], F32)
                    cout = dp.tile([64, 1], F32)
                    nc.sync.dma_start(cin[:], loc[:])
                    nc.gpsimd.collective_compute(
                        "AllReduce", OP.add,
                        replica_groups=[list(range(num_devices))],
                        ins=[cin.opt()], outs=[cout.opt()])
                    S128 = cp.tile([128, 1], F32)
                    nc.sync.dma_start(S128[0:64, :], cout[:])
                    nc.gpsimd.dma_start(S128[64:128, :], cout[:])
                    nc.vector.tensor_scalar(out=mu[:], in0=S128[:],
                                            scalar1=1.0 / float(N_TOTAL),
                                            scalar2=None, op0=OP.mult)
                else:
                    # mu = ((0.5*sum + N_CORE/2) * NCORES) / N_TOTAL
                    nc.vector.tensor_scalar(
                        out=mu[:], in0=s128[:],
                        scalar1=0.5 * NCORES / float(N_TOTAL),
                        scalar2=N_CORE * 0.5 * NCORES / float(N_TOTAL),
                        op0=OP.mult, op1=OP.add)
                # x = mu*(1-mu) + eps
                m1 = cp.tile([128, 1], F32)
                nc.vector.tensor_scalar(out=m1[:], in0=mu[:], scalar1=-1.0,
                                        scalar2=1.0, op0=OP.mult, op1=OP.add)
                x = cp.tile([128, 1], F32)
                nc.vector.tensor_tensor(x[:], m1[:], mu[:], OP.mult)
                nc.vector.tensor_scalar(out=x[:], in0=x[:], scalar1=EPS,
                                        scalar2=None, op0=OP.add)
                # r = 1/sqrt(x) + one Newton step r *= 1.5-0.5*x*r^2
                sq = cp.tile([128, 1], F32)
                nc.scalar.activation(sq[:], x[:], AF.Sqrt)
                r0 = cp.tile([128, 1], F32)
                nc.vector.reciprocal(r0[:], sq[:])
                e = cp.tile([128, 1], F32)
                nc.vector.tensor_tensor(e[:], r0[:], r0[:], OP.mult)
                nc.vector.tensor_tensor(e[:], e[:], x[:], OP.mult)
                nc.vector.tensor_scalar(out=e[:], in0=e[:], scalar1=-0.5,
                                        scalar2=1.5, op0=OP.mult, op1=OP.add)
                r = cp.tile([128, 1], F32)
                nc.vector.tensor_tensor(r[:], r0[:], e[:], OP.mult)
                # a = gamma*r ; scale = a/2 ; bias = a/2 + beta - mu*a
                a = cp.tile([128, 1], F32)
                nc.vector.tensor_tensor(a[:], gam_t, r[:], OP.mult)
                sc128 = cp.tile([128, 1], F32)
                nc.vector.tensor_scalar(out=sc128[:], in0=a[:], scalar1=0.5,
                                        scalar2=None, op0=OP.mult)
                tmp = cp.tile([128, 1], F32)
                nc.vector.tensor_tensor(tmp[:], mu[:], a[:], OP.mult)
                b0 = cp.tile([128, 1], F32)
                nc.vector.tensor_tensor(b0[:], bet_t, tmp[:], OP.subtract)
                bi128 = cp.tile([128, 1], F32)
                nc.vector.tensor_tensor(bi128[:], sc128[:], b0[:], OP.add)

                # ---------------- pass 2 ----------------
                for pair in range(NPAIR):
                    ot = op_.tile([128, T, BL], F16)
                    nc.vector.tensor_scalar(
                        out=ot[:], in0=store[:, pair * T:(pair + 1) * T, :],
                        scalar1=sc128[:], scalar2=bi128[:],
                        op0=OP.mult, op1=OP.add)
                    eng = (nc.sync, nc.gpsimd, nc.scalar)[pair % 3]
                    eng.dma_start(out_d[pair], ot[:])

    nc.compile()
    return nc


def _prep_host(dec, enc, Wg, bg, gamma, beta):
    Wg = np.asarray(Wg, dtype=np.float32)
    wdT = np.ascontiguousarray(Wg[:, :64].T)   # [k, m] dec-part
    weT = np.ascontiguousarray(Wg[:, 64:].T)   # enc-part
    wd = np.zeros((128, 128), dtype=np.float32)
    we = np.zeros((128, 128), dtype=np.float32)
    wd[:64, :64] = wdT
    wd[64:, 64:] = wdT
    we[:64, :64] = weT
    we[64:, 64:] = weT
    bgh = np.tile(0.5 * np.asarray(bg, np.float32), 2)
    idp = np.eye(128, dtype=np.float32) * 0.5
    idm = np.eye(128, dtype=np.float32) * -0.5

    def relayout(x):
        # [T, C, NPIX] -> [pair, p2*64+c, t, x448]
        x = np.asarray(x, np.float32).reshape(T, C, NPAIR, 2, BL)
        return np.ascontiguousarray(x.transpose(2, 3, 1, 0, 4)
                                    .reshape(NPAIR, 128, T, BL))
    par = np.zeros((128, 516), dtype=np.float32)
    par[:, 0:128] = wd
    par[:, 128:256] = we
    par[:, 256:384] = idp
    par[:, 384:512] = idm
    par[:, 512] = bgh
    par[:, 513] = -TH
    par[:, 514] = np.tile(np.asarray(gamma, np.float32), 2)
    par[:, 515] = np.tile(np.asarray(beta, np.float32), 2)
    in_maps = []
    for b in range(NCORES):
        in_maps.append({
            "dec": relayout(np.asarray(dec[:, b]).reshape(T, C, NPIX)),
            "enc": relayout(np.asarray(enc[:, b]).reshape(T, C, NPIX)),
            "par": par,
        })
    return in_maps


def kernel(dec, enc, Wg, bg, gamma, beta, _trace=False, _trace_kwargs=None):
    from concourse.bass_utils import run_bass_kernel_spmd

    if "nc" not in _cache:
        _cache["nc"] = _build()
    nc = _cache["nc"]

    in_maps = _prep_host(dec, enc, Wg, bg, gamma, beta)
    kw = {}
    if _trace:
        kw["trace"] = True
        if _trace_kwargs:
            kw.update(_trace_kwargs)
    res = run_bass_kernel_spmd(nc, in_maps, core_ids=list(range(NCORES)), **kw)
    outs = []
    for b in range(NCORES):
        o = np.asarray(res.results[b]["out"]).astype(np.float32)
        # [pair, p2*64+c, t, x448] -> [T, C, NPIX]
        o = o.reshape(NPAIR, 2, C, T, BL).transpose(3, 2, 0, 1, 4)
        outs.append(o.reshape(T, C, NPIX))
    out = np.stack(outs, axis=1).reshape(T, B, C, H, W)
    if _trace:
        _cache["last_res"] = res
    return out
